# revision 10
# baseline (speedup 1.0000x reference)
"""Trainium2 Bass kernel for EnhancedMPGNN (attention GNN + edge MLP).

Strategy (edge-parallel by destination column):
  - 500k random edges sharded across 8 cores by col ownership (6250 cols/core).
    Self-loop edges handled analytically node-parallel (no gathers).
  - Node tables replicated per layer via AllGather of [x_hat | lin(x)] slices.
  - Per-edge gathers via dma_gather (custom SWDGE instruction), cosine sim from
    pre-normalized x_hat, global softmax denominator via scalar AllReduce.
  - Scatter-add realized as one-hot matmuls accumulating in PSUM per 128-col
    group; aggregate stays in SBUF (no scatter instruction at all).
  - Final edge MLP: ep1 split into per-user/per-movie node projections
    (linearity), per-edge gathers + small matmuls.
"""
import sys
import numpy as np

sys.path.insert(0, "/opt/trn_rl_repo")

import concourse.bass as bass
import concourse.bacc as bacc
import concourse.tile as tile
import concourse.mybir as mybir
from concourse.bass_utils import run_bass_kernel_spmd
from concourse.library_config import mlp as mlp_lib

F32 = mybir.dt.float32
BF16 = mybir.dt.bfloat16
I16 = mybir.dt.int16
NC = 8
H = 128
L = 3
LN_EPS = 1e-5
COS_EPS = 1e-8
NEG = -30000.0  # exp(x+NEG) == 0.0 in f32
Relu = mybir.ActivationFunctionType.Relu
Exp = mybir.ActivationFunctionType.Exp
Sqrt = mybir.ActivationFunctionType.Sqrt
Square = mybir.ActivationFunctionType.Square
Copy = mybir.ActivationFunctionType.Copy
ADD = mybir.AluOpType.add
SUB = mybir.AluOpType.subtract
MUL = mybir.AluOpType.mult
MAX = mybir.AluOpType.max
ISEQ = mybir.AluOpType.is_equal


def _wrap_idx(seg):
    """int16 list (len%16==0) -> [16, n/16] wrapped block."""
    n = len(seg)
    return np.asarray(seg, np.int16).reshape(n // 16, 16).T


class Sched:
    pass


def _prep(x_user, x_movie, edge_index, movie_popularity, params):
    """Host-side sharding/scheduling. Returns (sched, in_maps, assemble)."""
    NUSER = x_user.shape[0]
    NMOV = x_movie.shape[0]
    E = edge_index.shape[1]
    row = np.asarray(edge_index[0], np.int64)
    col = np.asarray(edge_index[1], np.int64)
    UC = NUSER // NC              # owned user cols per core
    G = (UC + 127) // 128         # 128-col groups per core (= node tiles/side)
    UCP = G * 128                 # padded rows per core slice
    HALF = (NC // 2) * UCP        # row-gather table half size
    assert HALF < 32768 and UCP < 32768

    def r_of(n):                  # user node -> table row
        return UCP * (n // UC) + n % UC

    rtab = r_of(row)
    lo_edge = rtab < HALF
    core_of = col // UC

    # ---- edge phase schedule: per (core, group, half) edge lists ----
    per = [[[None, None] for _ in range(G)] for _ in range(NC)]
    for c in range(NC):
        m = np.nonzero(core_of == c)[0]
        gg = np.minimum((col[m] - c * UC) // 128, G - 1)
        lo = lo_edge[m]
        for g in range(G):
            sel = m[(gg == g)]
            losel = lo[(gg == g)]
            per[c][g][0] = sel[losel]
            per[c][g][1] = sel[~losel]
    TL = np.zeros((G, 2), np.int64)  # tiles per (group, half), shared
    for g in range(G):
        for h in range(2):
            mx = max(len(per[c][g][h]) for c in range(NC))
            TL[g, h] = max(1, (mx + 127) // 128)

    # superblocks of 2 groups
    SBS = [list(range(i, min(i + 2, G))) for i in range(0, G, 2)]
    sb_info = []
    roff = 0      # idx column offset (16 idx per column)
    gcol = 0      # global tile column (collocal / masklog)
    for groups in SBS:
        info = {"groups": groups, "nt": [0, 0], "roff": [0, 0],
                "tiles": [[], []]}
        for h in range(2):
            nt = int(sum(TL[g, h] for g in groups))
            info["nt"][h] = nt
            info["roff"][h] = roff
            bt = 0
            for g in groups:
                for _ in range(int(TL[g, h])):
                    info["tiles"][h].append((bt, g, gcol))
                    bt += 1
                    gcol += 1
            roff += nt * 8
        sb_info.append(info)
    TT = gcol
    RTOT = roff * 16            # total lanes

    # per-core lane arrays for edge phase
    rowidx = np.zeros((NC, 128, roff), np.int16)
    dstidx = np.zeros((NC, 128, roff), np.int16)
    collocal = np.zeros((NC, 128, TT), np.float32)
    masklog = np.full((NC, 128, TT), NEG, np.float32)
    for c in range(NC):
        for info in sb_info:
            for h in range(2):
                nt = info["nt"][h]
                if nt == 0:
                    continue
                lanes_r = np.zeros(nt * 128, np.int16)
                lanes_d = np.zeros(nt * 128, np.int16)
                pos = 0
                for g in info["groups"]:
                    tg = int(TL[g, h])
                    e = per[c][g][h]
                    k = len(e)
                    sl = slice(pos * 128, pos * 128 + k)
                    lanes_r[sl] = (rtab[e] - h * HALF).astype(np.int16)
                    lanes_d[sl] = (col[e] - c * UC).astype(np.int16)
                    # per-tile metadata columns
                    for t in range(tg):
                        gc = info["tiles"][h][pos + t][2]
                        lo_l = t * 128
                        hi_l = min(k, lo_l + 128)
                        nval = max(0, hi_l - lo_l)
                        if nval > 0:
                            cl = col[e[lo_l:hi_l]] - c * UC - 128 * g
                            collocal[c, :nval, gc] = cl.astype(np.float32)
                            masklog[c, :nval, gc] = 0.0
                    pos += tg
                o = info["roff"][h]
                rowidx[c, :, o:o + nt * 8] = np.tile(
                    _wrap_idx(lanes_r), (8, 1)).reshape(128, nt * 8)
                dstidx[c, :, o:o + nt * 8] = np.tile(
                    _wrap_idx(lanes_d), (8, 1)).reshape(128, nt * 8)

    # ---- self masks ----
    flat = np.arange(UCP)
    sm = np.where(flat < UC, 0.0, NEG).astype(np.float32)
    selfmask = sm.reshape(G, 128).T.copy()  # [128, G]

    # ---- final MLP schedule: 4 segments by (uhalf, mhalf) ----
    # pm is gathered from the core's PRIVATE pp_in (its own movie nodes),
    # so only the pu table half splits lanes: 2 segments by uhalf.
    NSEG = 2
    segid = (rtab >= HALF).astype(np.int64)
    fseg = [[None] * NSEG for _ in range(NC)]
    FS = np.zeros(NSEG, np.int64)
    for c in range(NC):
        m = np.nonzero(core_of == c)[0]
        for s in range(NSEG):
            fseg[c][s] = m[segid[m] == s]
            FS[s] = max(FS[s], (len(fseg[c][s]) + 127) // 128)
    FS = np.maximum(FS, 1)
    # blocks of up to 16 tiles per segment
    fblocks = []   # (seg, ntiles, lane_off)
    lane_off = 0
    for s in range(NSEG):
        t = 0
        while t < FS[s]:
            bt = int(min(16, FS[s] - t))
            fblocks.append((s, bt, lane_off))
            lane_off += bt * 128
            t += bt
    FTOT = ((lane_off + 4095) // 4096) * 4096
    fuidx = np.zeros((NC, 128, FTOT // 16), np.int16)
    fmidx = np.zeros((NC, 128, FTOT // 16), np.int16)
    fpop = np.zeros((NC, 1, FTOT), np.float32)
    edge_of_lane = np.full((NC, FTOT), -1, np.int64)
    fill_pos = np.zeros((NC, 4), np.int64)
    seg_base = {}
    off = 0
    for s in range(NSEG):
        seg_base[s] = off
        off += int(FS[s]) * 128
    pop = np.asarray(movie_popularity, np.float32)
    for c in range(NC):
        lanes_u = np.zeros(FTOT, np.int16)
        lanes_m = np.zeros(FTOT, np.int16)
        for s in range(NSEG):
            e = fseg[c][s]
            b = seg_base[s]
            lanes_u[b:b + len(e)] = (rtab[e] - s * HALF).astype(np.int16)
            lanes_m[b:b + len(e)] = (col[e] - c * UC).astype(np.int16)
            fpop[c, 0, b:b + len(e)] = pop[e]
            edge_of_lane[c, b:b + len(e)] = e
        # wrap per block
        for s, bt, lo in fblocks:
            n = bt * 128
            fuidx[c, :, lo // 16:(lo + n) // 16] = np.tile(
                _wrap_idx(lanes_u[lo:lo + n]), (8, 1)).reshape(128, n // 16)
            fmidx[c, :, lo // 16:(lo + n) // 16] = np.tile(
                _wrap_idx(lanes_m[lo:lo + n]), (8, 1)).reshape(128, n // 16)

    # ---- encoder inputs (transposed, padded, per core) ----
    FU = x_user.shape[1]
    FM = x_movie.shape[1]
    xuT = np.zeros((NC, FU, UCP), np.float32)
    xmT = np.zeros((NC, FM, UCP), np.float32)
    for c in range(NC):
        xuT[c, :, :UC] = np.asarray(x_user[c * UC:(c + 1) * UC], np.float32).T
        xmT[c, :, :UC] = np.asarray(x_movie[c * UC:(c + 1) * UC], np.float32).T

    # ---- weights (host-transposed / replicated) ----
    P = params
    rep = lambda v: np.tile(np.asarray(v, np.float32).reshape(1, -1), (128, 1))
    W = {}
    ue, me = P["user_enc"], P["movie_enc"]
    W["w1uT"] = np.asarray(ue["l1"]["w"], np.float32).T.copy()   # [32,128]
    W["b1u"] = rep(ue["l1"]["b"])
    W["w2uT"] = np.asarray(ue["l2"]["w"], np.float32).T.copy()
    W["b2u"] = rep(ue["l2"]["b"])
    W["gu"] = rep(ue["g"]); W["beu"] = rep(ue["beta"])
    W["w1mT"] = np.asarray(me["l1"]["w"], np.float32).T.copy()   # [64,128]
    W["b1m"] = rep(me["l1"]["b"])
    W["w2mT"] = np.asarray(me["l2"]["w"], np.float32).T.copy()
    W["b2m"] = rep(me["l2"]["b"])
    W["gm"] = rep(me["g"]); W["bem"] = rep(me["beta"])
    for i in range(L):
        cv = P["conv"][i]
        W[f"wc{i}T"] = np.asarray(cv["lin"]["w"], np.float32).T.copy()
        W[f"bc{i}"] = rep(cv["lin"]["b"])
        W[f"cg{i}"] = rep(cv["g"]); W[f"cb{i}"] = rep(cv["beta"])
    for i in range(L - 1):
        sk = P["skip"][i]
        W[f"ws{i}T"] = np.asarray(sk["w"], np.float32).T.copy()
        W[f"bs{i}"] = rep(sk["b"])
    ep1w = np.asarray(P["ep1"]["w"], np.float32)   # [128, 256]
    W["epuT"] = ep1w[:, :H].T.copy()
    W["epmT"] = ep1w[:, H:].T.copy()
    W["epb1"] = rep(P["ep1"]["b"])
    W["ep2T"] = np.asarray(P["ep2"]["w"], np.float32).T.copy()   # [128, 64]
    W["ep2b"] = np.asarray(P["ep2"]["b"], np.float32).reshape(64, 1).copy()
    W["ep3w"] = np.asarray(P["ep3"]["w"], np.float32).reshape(64, 1).copy()
    ep3b = float(np.asarray(P["ep3"]["b"]).reshape(()))
    popw = float(np.asarray(P["pop"]["w"]).reshape(()))
    popb = float(np.asarray(P["pop"]["b"]).reshape(())) + ep3b
    W["pwpb"] = np.array([[popw, popb]], np.float32)
    W["iota"] = np.tile(np.arange(128, dtype=np.float32), (128, 1)).copy()
    W["ident"] = np.eye(128, dtype=np.float32)
    W["ones_c"] = np.ones((128, 1), np.float32)
    W["ones_r"] = np.ones((1, 128), np.float32)
    W["selfmask"] = selfmask

    sched = Sched()
    sched.UC, sched.G, sched.UCP, sched.HALF = UC, G, UCP, HALF
    sched.TL, sched.sb_info, sched.TT, sched.RCOLS = TL, sb_info, TT, roff
    sched.FS, sched.fblocks, sched.FTOT = FS, fblocks, FTOT
    sched.FU, sched.FM = FU, FM
    sched.NSBH = 2 * len(sb_info)
    sched.key = (UC, G, TT, roff, FTOT, tuple(TL.ravel()),
                 tuple(int(x) for s in fblocks for x in s))

    in_maps = []
    for c in range(NC):
        d = dict(W)
        d["xuT"] = xuT[c]; d["xmT"] = xmT[c]
        d["rowidx"] = rowidx[c]; d["dstidx"] = dstidx[c]
        d["collocal"] = collocal[c]; d["masklog"] = masklog[c]
        d["fuidx"] = fuidx[c]; d["fmidx"] = fmidx[c]
        d["fpop"] = fpop[c]
        in_maps.append(d)

    assemble = (E, edge_of_lane)
    return sched, in_maps, assemble


def _ln_relu(nc, pool, v, g_rep, b_rep, do_relu=True):
    """LayerNorm along free dim of [128,128] tile v (sbuf) -> new sbuf tile."""
    musum = pool.tile([128, 1], F32, tag="ln_musum")
    nc.vector.tensor_reduce(musum[:], v[:], axis=mybir.AxisListType.X, op=ADD)
    mu = pool.tile([128, 1], F32, tag="ln_mu")
    nc.scalar.activation(mu[:], musum[:], Copy, scale=1.0 / H)
    xc = pool.tile([128, H], F32, tag="ln_xc")
    nc.vector.tensor_scalar(out=xc[:], in0=v[:], scalar1=mu[:], scalar2=None,
                            op0=SUB)
    sq = pool.tile([128, H], F32, tag="ln_sq")
    vs = pool.tile([128, 1], F32, tag="ln_vs")
    nc.scalar.activation(sq[:], xc[:], Square, accum_out=vs[:])
    var = pool.tile([128, 1], F32, tag="ln_var")
    nc.vector.tensor_scalar(out=var[:], in0=vs[:], scalar1=1.0 / H,
                            scalar2=LN_EPS, op0=MUL, op1=ADD)
    sd = pool.tile([128, 1], F32, tag="ln_sd")
    nc.scalar.activation(sd[:], var[:], Sqrt)
    rstd = pool.tile([128, 1], F32, tag="ln_rstd")
    nc.vector.reciprocal(rstd[:], sd[:])
    xn = pool.tile([128, H], F32, tag="ln_xn")
    nc.vector.scalar_tensor_tensor(out=xn[:], in0=xc[:], scalar=rstd[:],
                                   in1=g_rep[:], op0=MUL, op1=MUL)
    y = pool.tile([128, H], F32, tag="ln_y")
    if do_relu:
        nc.vector.tensor_add(xn[:], xn[:], b_rep[:])
        nc.scalar.activation(y[:], xn[:], Relu)
    else:
        nc.vector.tensor_add(y[:], xn[:], b_rep[:])
    return y


def build(sched):
    G, UCP, HALF, TT = sched.G, sched.UCP, sched.HALF, sched.TT
    RCOLS = sched.RCOLS
    NTMAX = max(max(i["nt"]) for i in sched.sb_info)
    nc = bacc.Bacc("TRN2", target_bir_lowering=False, debug=False,
                   num_devices=NC)

    # ---------------- tensors ----------------
    inp = {}
    def add_in(name, shape, dt=F32):
        inp[name] = nc.dram_tensor(name, list(shape), dt, kind="ExternalInput")
    add_in("xuT", (sched.FU, UCP)); add_in("xmT", (sched.FM, UCP))
    add_in("rowidx", (128, RCOLS), I16); add_in("dstidx", (128, RCOLS), I16)
    add_in("collocal", (128, TT)); add_in("masklog", (128, TT))
    add_in("fuidx", (128, sched.FTOT // 16), I16)
    add_in("fmidx", (128, sched.FTOT // 16), I16)
    add_in("fpop", (1, sched.FTOT))
    for nm in ["w1uT", "b1u", "w2uT", "b2u", "gu", "beu",
               "w1mT", "b1m", "w2mT", "b2m", "gm", "bem",
               "epuT", "epmT", "epb1", "iota", "ident", "selfmask"]:
        add_in(nm, {"w1uT": (sched.FU, 128), "w1mT": (sched.FM, 128),
                    "selfmask": (128, G)}.get(nm, (128, 128)))
    for i in range(L):
        add_in(f"wc{i}T", (128, 128)); add_in(f"bc{i}", (128, 128))
        add_in(f"cg{i}", (128, 128)); add_in(f"cb{i}", (128, 128))
    for i in range(L - 1):
        add_in(f"ws{i}T", (128, 128)); add_in(f"bs{i}", (128, 128))
    add_in("ep2T", (128, 64)); add_in("ep2b", (64, 1)); add_in("ep3w", (64, 1))
    add_in("pwpb", (1, 2)); add_in("ones_c", (128, 1)); add_in("ones_r", (1, 128))

    out_dram = nc.dram_tensor("out", [1, sched.FTOT], F32, kind="ExternalOutput")

    ag_in = nc.dram_tensor("ag_in", [UCP, 256], F32)
    utable = nc.dram_tensor("utable", [NC * UCP, 256], F32, addr_space="Shared")
    mslice = nc.dram_tensor("mslice", [UCP, 256], F32)
    presk_u = nc.dram_tensor("presk_u", [UCP, H], F32)
    presk_m = nc.dram_tensor("presk_m", [UCP, H], F32)
    pp_in = nc.dram_tensor("pp_in", [UCP, 256], F32)
    pptable = nc.dram_tensor("pptable", [NC * UCP, 256], F32, addr_space="Shared")
    dbounce = nc.dram_tensor("dbounce", [1, 8], F32)
    dout_t = nc.dram_tensor("dout", [1, 8], F32, addr_space="Shared")

    from contextlib import ExitStack
    with tile.TileContext(nc) as tc, ExitStack() as ctx:
        nc.gpsimd.load_library(mlp_lib)
        cp = ctx.enter_context(tc.tile_pool(name="consts", bufs=1))
        C = {}
        for nm, t in inp.items():
            if nm in ("xuT", "xmT", "fpop"):
                continue
            C[nm] = cp.tile(list(t.shape), t.dtype, tag=f"c_{nm}", name=f"c_{nm}")
            nc.sync.dma_start(C[nm][:], t[:])

        wp = ctx.enter_context(tc.tile_pool(name="work", bufs=3))
        pp = ctx.enter_context(tc.tile_pool(name="psumw", bufs=1, space="PSUM"))

        # x tile [128n, 128h] -> writes slice_dram rows [x_hat | lin+b]; also
        # extra matmuls (name, wT, bias, dst, dstcol) using transposed x.
        def finish_node(xt, g, slice_dram, lin_w, lin_b, extras):
            sq = wp.tile([128, H], F32, tag="fn_sq")
            ss = wp.tile([128, 1], F32, tag="fn_ss")
            nc.scalar.activation(sq[:], xt[:], Square, accum_out=ss[:])
            nrm = wp.tile([128, 1], F32, tag="fn_nrm")
            nc.scalar.activation(nrm[:], ss[:], Sqrt)
            nc.vector.tensor_scalar(out=nrm[:], in0=nrm[:], scalar1=COS_EPS,
                                    scalar2=None, op0=MAX)
            q = wp.tile([128, 1], F32, tag="fn_q")
            nc.vector.reciprocal(q[:], nrm[:])
            xh = wp.tile([128, H], F32, tag="fn_xh")
            nc.vector.tensor_scalar(out=xh[:], in0=xt[:], scalar1=q[:],
                                    scalar2=None, op0=MUL)
            if slice_dram is not None:
                nc.sync.dma_start(slice_dram[g * 128:(g + 1) * 128, 0:H], xh[:])
            pt = pp.tile([128, H], F32, tag="ps", bufs=3)
            nc.tensor.transpose(pt[:], xt[:], C["ident"][:])
            xT = wp.tile([128, H], F32, tag="fn_xT")
            nc.vector.tensor_copy(xT[:], pt[:])
            if lin_w is not None:
                pl = pp.tile([128, H], F32, tag="ps", bufs=3)
                nc.tensor.matmul(pl[:], xT[:], lin_w[:], start=True, stop=True)
                lx = wp.tile([128, H], F32, tag="fn_lx")
                nc.vector.tensor_add(lx[:], pl[:], lin_b[:])
                nc.sync.dma_start(slice_dram[g * 128:(g + 1) * 128, H:256], lx[:])
            for (wT, bias, dst, c0) in extras:
                pe = pp.tile([128, H], F32, tag="ps", bufs=3)
                nc.tensor.matmul(pe[:], xT[:], wT[:], start=True, stop=True)
                ex = wp.tile([128, H], F32, tag="fn_ex")
                if bias is not None:
                    nc.vector.tensor_add(ex[:], pe[:], bias[:])
                else:
                    nc.vector.tensor_copy(ex[:], pe[:])
                nc.sync.dma_start(dst[g * 128:(g + 1) * 128, c0:c0 + H], ex[:])

        # ---------------- encoders ----------------
        with tc.tile_pool(name="enc", bufs=3) as ep:
            xuT_sb = ep.tile([sched.FU, UCP], F32, tag="xuT", bufs=1)
            nc.sync.dma_start(xuT_sb[:], inp["xuT"][:])
            xmT_sb = ep.tile([sched.FM, UCP], F32, tag="xmT", bufs=1)
            nc.sync.dma_start(xmT_sb[:], inp["xmT"][:])
            for side in range(2):
                xT_in = xuT_sb if side == 0 else xmT_sb
                w1 = C["w1uT"] if side == 0 else C["w1mT"]
                b1 = C["b1u"] if side == 0 else C["b1m"]
                w2 = C["w2uT"] if side == 0 else C["w2mT"]
                b2 = C["b2u"] if side == 0 else C["b2m"]
                gr = C["gu"] if side == 0 else C["gm"]
                br = C["beu"] if side == 0 else C["bem"]
                sl = ag_in if side == 0 else mslice
                for g in range(G):
                    p1 = pp.tile([128, H], F32, tag="ps", bufs=3)
                    nc.tensor.matmul(p1[:], xT_in[:, g * 128:(g + 1) * 128],
                                     w1[:], start=True, stop=True)
                    h1 = ep.tile([128, H], F32, tag="enc_h1")
                    nc.vector.tensor_add(h1[:], p1[:], b1[:])
                    h1r = ep.tile([128, H], F32, tag="enc_h1r")
                    nc.scalar.activation(h1r[:], h1[:], Relu)
                    pt = pp.tile([128, H], F32, tag="ps", bufs=3)
                    nc.tensor.transpose(pt[:], h1r[:], C["ident"][:])
                    h1T = ep.tile([128, H], F32, tag="enc_h1T")
                    nc.vector.tensor_copy(h1T[:], pt[:])
                    p2 = pp.tile([128, H], F32, tag="ps", bufs=3)
                    nc.tensor.matmul(p2[:], h1T[:], w2[:], start=True, stop=True)
                    z = ep.tile([128, H], F32, tag="enc_z")
                    nc.vector.tensor_add(z[:], p2[:], b2[:])
                    x0 = _ln_relu(nc, ep, z, gr, br, do_relu=False)
                    finish_node(x0, g, sl, C["wc0T"], C["bc0"], [])

        # ---------------- layers ----------------
        for l in range(L):
            with tc.tile_pool(name=f"edge{l}", bufs=2) as xp, \
                 tc.tile_pool(name=f"agg{l}", bufs=1) as ap, \
                 tc.tile_pool(name=f"oh{l}", bufs=4) as ohp, \
                 tc.tile_pool(name=f"eps{l}", bufs=1, space="PSUM") as eps:
                nc.gpsimd.collective_compute(
                    "AllGather", mybir.AluOpType.bypass,
                    replica_groups=[list(range(NC))],
                    ins=[ag_in[:]], outs=[utable[:]])

                aggr = ap.tile([128, G, H], F32, tag="aggr")
                dstage = ap.tile([128, sched.NSBH], F32, tag="dstage")
                wsu = ap.tile([128, G], F32, tag="wsu")
                wsm = ap.tile([128, G], F32, tag="wsm")

                k_sbh = 0
                for info in sched.sb_info:
                    bufs = [None, None]
                    whs = [None, None]
                    for h2 in range(2):
                        nt = info["nt"][h2]
                        if nt == 0:
                            k_sbh += 1
                            continue
                        rb = xp.tile([128, NTMAX, 256], F32, tag="rowbuf")
                        view = utable[h2 * HALF:(h2 + 1) * HALF, :]
                        o = info["roff"][h2]
                        nc.gpsimd.dma_gather(
                            out_ap=rb[:, 0:nt, :], in_ap=view,
                            idxs_ap=C["rowidx"][:, o:o + nt * 8],
                            num_idxs=nt * 128, num_idxs_reg=nt * 128,
                            elem_size=256, single_packet=False)
                        db = xp.tile([128, NTMAX, H], F32, tag="dstbuf")
                        nc.gpsimd.dma_gather(
                            out_ap=db[:, 0:nt, :], in_ap=ag_in[:, 0:H],
                            idxs_ap=C["dstidx"][:, o:o + nt * 8],
                            num_idxs=nt * 128, num_idxs_reg=nt * 128,
                            elem_size=H, elem_step=256, single_packet=False)
                        nc.vector.tensor_mul(db[:, 0:nt, :],
                                             rb[:, 0:nt, 0:H], db[:, 0:nt, :])
                        dots = xp.tile([128, NTMAX], F32, tag="dots")
                        nc.vector.tensor_reduce(
                            dots[:, 0:nt], db[:, 0:nt, :],
                            axis=mybir.AxisListType.X, op=ADD)
                        gc0 = info["tiles"][h2][0][2]
                        nc.vector.tensor_add(dots[:, 0:nt], dots[:, 0:nt],
                                             C["masklog"][:, gc0:gc0 + nt])
                        wh = xp.tile([128, NTMAX], F32, tag="wh")
                        nc.scalar.activation(wh[:, 0:nt], dots[:, 0:nt], Exp,
                                             accum_out=dstage[:, k_sbh:k_sbh + 1])
                        bufs[h2] = rb
                        whs[h2] = wh
                        k_sbh += 1
                    for g in info["groups"]:
                        chain = []
                        for h2 in range(2):
                            for (bt, gg, gc) in info["tiles"][h2]:
                                if gg == g:
                                    chain.append((h2, bt, gc))
                        pg = eps.tile([128, H], F32, tag="pgroup", bufs=2)
                        for i, (h2, bt, gc) in enumerate(chain):
                            oh = ohp.tile([128, 128], F32, tag="oh")
                            nc.vector.tensor_scalar(
                                out=oh[:], in0=C["iota"][:],
                                scalar1=C["collocal"][:, gc:gc + 1],
                                scalar2=whs[h2][:, bt:bt + 1],
                                op0=ISEQ, op1=MUL)
                            nc.tensor.matmul(
                                pg[:], oh[:], bufs[h2][:, bt, H:256],
                                start=(i == 0), stop=(i == len(chain) - 1))
                        nc.vector.tensor_copy(aggr[:, g, :], pg[:])

                # self sims
                for side in range(2):
                    sl = ag_in if side == 0 else mslice
                    ws = wsu if side == 0 else wsm
                    for g in range(G):
                        ar = xp.tile([128, 256], F32, tag="selfrow")
                        nc.sync.dma_start(ar[:], sl[g * 128:(g + 1) * 128, :])
                        sq = xp.tile([128, H], F32, tag="self_sq")
                        ss = xp.tile([128, 1], F32, tag="self_ss")
                        nc.scalar.activation(sq[:], ar[:, 0:H], Square,
                                             accum_out=ss[:])
                        nc.scalar.activation(ws[:, g:g + 1], ss[:], Exp,
                                             bias=C["selfmask"][:, g:g + 1])

                # D total + allreduce
                d1 = xp.tile([128, 1], F32, tag="d1")
                nc.vector.tensor_reduce(d1[:], dstage[:],
                                        axis=mybir.AxisListType.X, op=ADD)
                d2 = xp.tile([128, 1], F32, tag="d2")
                nc.vector.tensor_reduce(d2[:], wsu[:],
                                        axis=mybir.AxisListType.X, op=ADD)
                d3 = xp.tile([128, 1], F32, tag="d3")
                nc.vector.tensor_reduce(d3[:], wsm[:],
                                        axis=mybir.AxisListType.X, op=ADD)
                nc.vector.tensor_add(d1[:], d1[:], d2[:])
                nc.vector.tensor_add(d1[:], d1[:], d3[:])
                pd = eps.tile([1, 1], F32, tag="pmisc", bufs=1)
                nc.tensor.matmul(pd[:], d1[:], C["ones_c"][:], start=True,
                                 stop=True)
                dsb = xp.tile([1, 1], F32, tag="dsb")
                nc.vector.tensor_copy(dsb[:], pd[:])
                nc.sync.dma_start(dbounce[0:1, 0:1], dsb[:])
                nc.gpsimd.collective_compute(
                    "AllReduce", ADD, replica_groups=[list(range(NC))],
                    ins=[dbounce[:]], outs=[dout_t[:]])
                dval = xp.tile([1, 1], F32, tag="dval")
                nc.sync.dma_start(dval[:], dout_t[0:1, 0:1])
                pb = eps.tile([128, 1], F32, tag="pmisc", bufs=1)
                nc.tensor.matmul(pb[:], C["ones_r"][:], dval[:], start=True,
                                 stop=True)
                recipD = ap.tile([128, 1], F32, tag="recipD")
                nc.vector.reciprocal(recipD[:], pb[:])

                # ---------------- node update ----------------
                last = (l == L - 1)
                for side in range(2):
                    sl = ag_in if side == 0 else mslice
                    psk = presk_u if side == 0 else presk_m
                    ws = wsu if side == 0 else wsm
                    for g in range(G):
                        ar = xp.tile([128, 256], F32, tag="nd_row")
                        nc.sync.dma_start(ar[:], sl[g * 128:(g + 1) * 128, :])
                        lx = ar[:, H:256]
                        t1 = xp.tile([128, H], F32, tag="nd_t1")
                        if side == 0:
                            nc.vector.scalar_tensor_tensor(
                                out=t1[:], in0=lx, scalar=ws[:, g:g + 1],
                                in1=aggr[:, g, :], op0=MUL, op1=ADD)
                        else:
                            nc.vector.tensor_scalar(
                                out=t1[:], in0=lx, scalar1=ws[:, g:g + 1],
                                scalar2=None, op0=MUL)
                        nc.vector.tensor_scalar(out=t1[:], in0=t1[:],
                                                scalar1=recipD[:],
                                                scalar2=None, op0=MUL)
                        v = xp.tile([128, H], F32, tag="nd_v")
                        nc.vector.tensor_add(v[:], t1[:], lx)
                        xr = _ln_relu(nc, xp, v, C[f"cg{l}"], C[f"cb{l}"],
                                      do_relu=True)
                        if l > 0:
                            pk = xp.tile([128, H], F32, tag="nd_pk")
                            nc.sync.dma_start(pk[:],
                                              psk[g * 128:(g + 1) * 128, :])
                            nc.vector.tensor_add(xr[:], xr[:], pk[:])
                        if not last:
                            extras = [(C[f"ws{l}T"], C[f"bs{l}"], psk, 0)]
                            finish_node(xr, g, sl, C[f"wc{l + 1}T"],
                                        C[f"bc{l + 1}"], extras)
                        else:
                            # pu for user slots, pm for movie slots; the other
                            # half of each pp_in row is never gathered.
                            ex = ([(C["epuT"], C["epb1"], pp_in, 0)]
                                  if side == 0 else
                                  [(C["epmT"], None, pp_in, H)])
                            finish_node(xr, g, None, None, None, ex)

        # ---------------- final MLP ----------------
        with tc.tile_pool(name="fin", bufs=2) as fp, \
             tc.tile_pool(name="fps", bufs=1, space="PSUM") as fps, \
             tc.tile_pool(name="fout", bufs=2) as fop:
            nc.gpsimd.collective_compute(
                "AllGather", mybir.AluOpType.bypass,
                replica_groups=[list(range(NC))],
                ins=[pp_in[:]], outs=[pptable[:]])
            CHUNK = 4096
            ost = None
            ost_base = 0
            for (s, bt, lo) in sched.fblocks:
                uh = s
                pu = fp.tile([128, 16, H], F32, tag="f_pu")
                nc.gpsimd.dma_gather(
                    out_ap=pu[:, 0:bt, :],
                    in_ap=pptable[uh * HALF:(uh + 1) * HALF, 0:H],
                    idxs_ap=C["fuidx"][:, lo // 16:(lo + bt * 128) // 16],
                    num_idxs=bt * 128, num_idxs_reg=bt * 128,
                    elem_size=H, elem_step=256, single_packet=False)
                pm = fp.tile([128, 16, H], F32, tag="f_pm")
                nc.gpsimd.dma_gather(
                    out_ap=pm[:, 0:bt, :],
                    in_ap=pp_in[:, H:256],
                    idxs_ap=C["fmidx"][:, lo // 16:(lo + bt * 128) // 16],
                    num_idxs=bt * 128, num_idxs_reg=bt * 128,
                    elem_size=H, elem_step=256, single_packet=False)
                h1 = fp.tile([128, 16, H], F32, tag="f_h1")
                nc.vector.tensor_add(h1[:, 0:bt, :], pu[:, 0:bt, :],
                                     pm[:, 0:bt, :])
                h1r = fp.tile([128, 16, H], F32, tag="f_h1r")
                nc.scalar.activation(h1r[:, 0:bt, :], h1[:, 0:bt, :], Relu)
                for t in range(bt):
                    ptr = fps.tile([128, H], F32, tag="f_tr", bufs=2)
                    nc.tensor.transpose(ptr[:], h1r[:, t, :], C["ident"][:])
                    h1T = fp.tile([128, H], F32, tag="f_h1T")
                    nc.vector.tensor_copy(h1T[:], ptr[:])
                    p2 = fps.tile([64, 128], F32, tag="f_p2", bufs=2)
                    nc.tensor.matmul(p2[:], C["ep2T"][:], h1T[:], start=True,
                                     stop=True)
                    h2r = fp.tile([64, 128], F32, tag="f_h2r")
                    nc.scalar.activation(h2r[:], p2[:], Relu,
                                         bias=C["ep2b"][:])
                    p3 = fps.tile([1, 128], F32, tag="f_p3", bufs=1)
                    nc.tensor.matmul(p3[:], C["ep3w"][:], h2r[:], start=True,
                                     stop=True)
                    lane0 = lo + t * 128
                    if ost is None or lane0 - ost_base >= CHUNK:
                        if ost is not None:
                            nc.sync.dma_start(
                                out_dram[0:1, ost_base:ost_base + CHUNK],
                                ost[:])
                        ost = fop.tile([1, CHUNK], F32, tag="f_ost", bufs=1)
                        ost_base = lane0
                        popc = fop.tile([1, CHUNK], F32, tag="f_popc", bufs=1)
                        nc.sync.dma_start(
                            popc[:], inp["fpop"][0:1, ost_base:ost_base + CHUNK])
                        nc.vector.tensor_scalar(
                            out=ost[:], in0=popc[:],
                            scalar1=C["pwpb"][0:1, 0:1],
                            scalar2=C["pwpb"][0:1, 1:2], op0=MUL, op1=ADD)
                    co = lane0 - ost_base
                    nc.vector.tensor_add(ost[0:1, co:co + 128],
                                         ost[0:1, co:co + 128], p3[:])
            if ost is not None:
                nc.sync.dma_start(out_dram[0:1, ost_base:ost_base + CHUNK],
                                  ost[:])

    nc.compile()
    return nc


_CACHE = {}
LAST_RESULT = None


def kernel(x_user, x_movie, edge_index, movie_popularity, params):
    x_user = np.asarray(x_user)
    x_movie = np.asarray(x_movie)
    edge_index = np.asarray(edge_index)
    movie_popularity = np.asarray(movie_popularity)
    sched, in_maps, (E, edge_of_lane) = _prep(
        x_user, x_movie, edge_index, movie_popularity, params)
    nc = _CACHE.get(sched.key)
    if nc is None:
        nc = build(sched)
        _CACHE[sched.key] = nc
    res = run_bass_kernel_spmd(nc, in_maps, core_ids=list(range(NC)))
    global LAST_RESULT
    LAST_RESULT = res
    y = np.zeros((E, 1), np.float32)
    for c in range(NC):
        lanes = edge_of_lane[c]
        m = lanes >= 0
        y[lanes[m], 0] = res.results[c]["out"][0][m]
    return y


# revision 14
# speedup vs baseline: 1.0024x; 1.0024x over previous
"""Trainium2 Bass kernel for EnhancedMPGNN (attention GNN + edge MLP).

Strategy (edge-parallel by destination column):
  - 500k random edges sharded across 8 cores by col ownership (6250 cols/core).
    Self-loop edges handled analytically node-parallel (no gathers).
  - Node tables replicated per layer via AllGather of [x_hat | lin(x)] slices.
  - Per-edge gathers via dma_gather (custom SWDGE instruction), cosine sim from
    pre-normalized x_hat, global softmax denominator via scalar AllReduce.
  - Scatter-add realized as one-hot matmuls accumulating in PSUM per 128-col
    group; aggregate stays in SBUF (no scatter instruction at all).
  - Final edge MLP: ep1 split into per-user/per-movie node projections
    (linearity), per-edge gathers + small matmuls.
"""
import sys
import numpy as np

sys.path.insert(0, "/opt/trn_rl_repo")

import concourse.bass as bass
import concourse.bacc as bacc
import concourse.tile as tile
import concourse.mybir as mybir
from concourse.bass_utils import run_bass_kernel_spmd
from concourse.library_config import mlp as mlp_lib

F32 = mybir.dt.float32
BF16 = mybir.dt.bfloat16
I16 = mybir.dt.int16
NC = 8
H = 128
L = 3
LN_EPS = 1e-5
COS_EPS = 1e-8
NEG = -30000.0  # exp(x+NEG) == 0.0 in f32
Relu = mybir.ActivationFunctionType.Relu
Exp = mybir.ActivationFunctionType.Exp
Sqrt = mybir.ActivationFunctionType.Sqrt
Square = mybir.ActivationFunctionType.Square
Copy = mybir.ActivationFunctionType.Copy
ADD = mybir.AluOpType.add
SUB = mybir.AluOpType.subtract
MUL = mybir.AluOpType.mult
MAX = mybir.AluOpType.max
ISEQ = mybir.AluOpType.is_equal


def _wrap_idx(seg):
    """int16 list (len%16==0) -> [16, n/16] wrapped block."""
    n = len(seg)
    return np.asarray(seg, np.int16).reshape(n // 16, 16).T


class Sched:
    pass


def _prep(x_user, x_movie, edge_index, movie_popularity, params):
    """Host-side sharding/scheduling. Returns (sched, in_maps, assemble)."""
    NUSER = x_user.shape[0]
    NMOV = x_movie.shape[0]
    E = edge_index.shape[1]
    row = np.asarray(edge_index[0], np.int64)
    col = np.asarray(edge_index[1], np.int64)
    UC = NUSER // NC              # owned user cols per core
    G = (UC + 127) // 128         # 128-col groups per core (= node tiles/side)
    UCP = G * 128                 # padded rows per core slice
    HALF = (NC // 2) * UCP        # row-gather table half size
    assert HALF < 32768 and UCP < 32768

    def r_of(n):                  # user node -> table row
        return UCP * (n // UC) + n % UC

    rtab = r_of(row)
    lo_edge = rtab < HALF
    core_of = col // UC

    # ---- edge phase schedule: per (core, group, half) edge lists ----
    per = [[[None, None] for _ in range(G)] for _ in range(NC)]
    for c in range(NC):
        m = np.nonzero(core_of == c)[0]
        gg = np.minimum((col[m] - c * UC) // 128, G - 1)
        lo = lo_edge[m]
        for g in range(G):
            sel = m[(gg == g)]
            losel = lo[(gg == g)]
            per[c][g][0] = sel[losel]
            per[c][g][1] = sel[~losel]
    TL = np.zeros((G, 2), np.int64)  # tiles per (group, half), shared
    for g in range(G):
        for h in range(2):
            mx = max(len(per[c][g][h]) for c in range(NC))
            TL[g, h] = max(1, (mx + 127) // 128)

    # superblocks of 2 groups
    SBS = [list(range(i, min(i + 2, G))) for i in range(0, G, 2)]
    sb_info = []
    roff = 0      # idx column offset (16 idx per column)
    gcol = 0      # global tile column (collocal / masklog)
    for groups in SBS:
        info = {"groups": groups, "nt": [0, 0], "roff": [0, 0],
                "tiles": [[], []]}
        for h in range(2):
            nt = int(sum(TL[g, h] for g in groups))
            info["nt"][h] = nt
            info["roff"][h] = roff
            bt = 0
            for g in groups:
                for _ in range(int(TL[g, h])):
                    info["tiles"][h].append((bt, g, gcol))
                    bt += 1
                    gcol += 1
            roff += nt * 8
        sb_info.append(info)
    TT = gcol
    RTOT = roff * 16            # total lanes

    # per-core lane arrays for edge phase
    rowidx = np.zeros((NC, 128, roff), np.int16)
    dstidx = np.zeros((NC, 128, roff), np.int16)
    collocal = np.zeros((NC, 128, TT), np.float32)
    masklog = np.full((NC, 128, TT), NEG, np.float32)
    for c in range(NC):
        for info in sb_info:
            for h in range(2):
                nt = info["nt"][h]
                if nt == 0:
                    continue
                lanes_r = np.zeros(nt * 128, np.int16)
                lanes_d = np.zeros(nt * 128, np.int16)
                pos = 0
                for g in info["groups"]:
                    tg = int(TL[g, h])
                    e = per[c][g][h]
                    k = len(e)
                    sl = slice(pos * 128, pos * 128 + k)
                    lanes_r[sl] = (rtab[e] - h * HALF).astype(np.int16)
                    lanes_d[sl] = (col[e] - c * UC).astype(np.int16)
                    # per-tile metadata columns
                    for t in range(tg):
                        gc = info["tiles"][h][pos + t][2]
                        lo_l = t * 128
                        hi_l = min(k, lo_l + 128)
                        nval = max(0, hi_l - lo_l)
                        if nval > 0:
                            cl = col[e[lo_l:hi_l]] - c * UC - 128 * g
                            collocal[c, :nval, gc] = cl.astype(np.float32)
                            masklog[c, :nval, gc] = 0.0
                    pos += tg
                o = info["roff"][h]
                rowidx[c, :, o:o + nt * 8] = np.tile(
                    _wrap_idx(lanes_r), (8, 1)).reshape(128, nt * 8)
                dstidx[c, :, o:o + nt * 8] = np.tile(
                    _wrap_idx(lanes_d), (8, 1)).reshape(128, nt * 8)

    # ---- self masks ----
    flat = np.arange(UCP)
    sm = np.where(flat < UC, 0.0, NEG).astype(np.float32)
    selfmask = sm.reshape(G, 128).T.copy()  # [128, G]

    # ---- final MLP schedule: 4 segments by (uhalf, mhalf) ----
    # pm is gathered from the core's PRIVATE pp_in (its own movie nodes),
    # so only the pu table half splits lanes: 2 segments by uhalf.
    NSEG = 2
    segid = (rtab >= HALF).astype(np.int64)
    fseg = [[None] * NSEG for _ in range(NC)]
    FS = np.zeros(NSEG, np.int64)
    for c in range(NC):
        m = np.nonzero(core_of == c)[0]
        for s in range(NSEG):
            fseg[c][s] = m[segid[m] == s]
            FS[s] = max(FS[s], (len(fseg[c][s]) + 127) // 128)
    FS = np.maximum(FS, 1)
    # blocks of up to 16 tiles per segment
    fblocks = []   # (seg, ntiles, lane_off)
    lane_off = 0
    for s in range(NSEG):
        t = 0
        while t < FS[s]:
            bt = int(min(16, FS[s] - t))
            fblocks.append((s, bt, lane_off))
            lane_off += bt * 128
            t += bt
    FTOT = ((lane_off + 4095) // 4096) * 4096
    fuidx = np.zeros((NC, 128, FTOT // 16), np.int16)
    fmidx = np.zeros((NC, 128, FTOT // 16), np.int16)
    fpop = np.zeros((NC, 1, FTOT), np.float32)
    edge_of_lane = np.full((NC, FTOT), -1, np.int64)
    fill_pos = np.zeros((NC, 4), np.int64)
    seg_base = {}
    off = 0
    for s in range(NSEG):
        seg_base[s] = off
        off += int(FS[s]) * 128
    pop = np.asarray(movie_popularity, np.float32)
    for c in range(NC):
        lanes_u = np.zeros(FTOT, np.int16)
        lanes_m = np.zeros(FTOT, np.int16)
        for s in range(NSEG):
            e = fseg[c][s]
            b = seg_base[s]
            lanes_u[b:b + len(e)] = (rtab[e] - s * HALF).astype(np.int16)
            lanes_m[b:b + len(e)] = (col[e] - c * UC).astype(np.int16)
            fpop[c, 0, b:b + len(e)] = pop[e]
            edge_of_lane[c, b:b + len(e)] = e
        # wrap per block
        for s, bt, lo in fblocks:
            n = bt * 128
            fuidx[c, :, lo // 16:(lo + n) // 16] = np.tile(
                _wrap_idx(lanes_u[lo:lo + n]), (8, 1)).reshape(128, n // 16)
            fmidx[c, :, lo // 16:(lo + n) // 16] = np.tile(
                _wrap_idx(lanes_m[lo:lo + n]), (8, 1)).reshape(128, n // 16)

    # ---- encoder inputs (transposed, padded, per core) ----
    FU = x_user.shape[1]
    FM = x_movie.shape[1]
    xuT = np.zeros((NC, FU, UCP), np.float32)
    xmT = np.zeros((NC, FM, UCP), np.float32)
    for c in range(NC):
        xuT[c, :, :UC] = np.asarray(x_user[c * UC:(c + 1) * UC], np.float32).T
        xmT[c, :, :UC] = np.asarray(x_movie[c * UC:(c + 1) * UC], np.float32).T

    # ---- weights (host-transposed / replicated) ----
    P = params
    rep = lambda v: np.tile(np.asarray(v, np.float32).reshape(1, -1), (128, 1))
    W = {}
    ue, me = P["user_enc"], P["movie_enc"]
    W["w1uT"] = np.asarray(ue["l1"]["w"], np.float32).T.copy()   # [32,128]
    W["b1u"] = rep(ue["l1"]["b"])
    W["w2uT"] = np.asarray(ue["l2"]["w"], np.float32).T.copy()
    W["b2u"] = rep(ue["l2"]["b"])
    W["gu"] = rep(ue["g"]); W["beu"] = rep(ue["beta"])
    W["w1mT"] = np.asarray(me["l1"]["w"], np.float32).T.copy()   # [64,128]
    W["b1m"] = rep(me["l1"]["b"])
    W["w2mT"] = np.asarray(me["l2"]["w"], np.float32).T.copy()
    W["b2m"] = rep(me["l2"]["b"])
    W["gm"] = rep(me["g"]); W["bem"] = rep(me["beta"])
    for i in range(L):
        cv = P["conv"][i]
        W[f"wc{i}T"] = np.asarray(cv["lin"]["w"], np.float32).T.copy()
        W[f"bc{i}"] = rep(cv["lin"]["b"])
        W[f"cg{i}"] = rep(cv["g"]); W[f"cb{i}"] = rep(cv["beta"])
    for i in range(L - 1):
        sk = P["skip"][i]
        W[f"ws{i}T"] = np.asarray(sk["w"], np.float32).T.copy()
        W[f"bs{i}"] = rep(sk["b"])
    ep1w = np.asarray(P["ep1"]["w"], np.float32)   # [128, 256]
    W["epuT"] = ep1w[:, :H].T.copy()
    W["epmT"] = ep1w[:, H:].T.copy()
    W["epb1"] = rep(P["ep1"]["b"])
    W["ep2T"] = np.asarray(P["ep2"]["w"], np.float32).T.copy()   # [128, 64]
    W["ep2b"] = np.asarray(P["ep2"]["b"], np.float32).reshape(64, 1).copy()
    W["ep3w"] = np.asarray(P["ep3"]["w"], np.float32).reshape(64, 1).copy()
    ep3b = float(np.asarray(P["ep3"]["b"]).reshape(()))
    popw = float(np.asarray(P["pop"]["w"]).reshape(()))
    popb = float(np.asarray(P["pop"]["b"]).reshape(())) + ep3b
    W["pwpb"] = np.array([[popw, popb]], np.float32)
    W["iota"] = np.tile(np.arange(128, dtype=np.float32), (128, 1)).copy()
    W["ident"] = np.eye(128, dtype=np.float32)
    W["ones_c"] = np.ones((128, 1), np.float32)
    W["ones_r"] = np.ones((1, 128), np.float32)
    W["selfmask"] = selfmask

    sched = Sched()
    sched.UC, sched.G, sched.UCP, sched.HALF = UC, G, UCP, HALF
    sched.TL, sched.sb_info, sched.TT, sched.RCOLS = TL, sb_info, TT, roff
    sched.FS, sched.fblocks, sched.FTOT = FS, fblocks, FTOT
    sched.FU, sched.FM = FU, FM
    sched.NSBH = 2 * len(sb_info)
    sched.key = (UC, G, TT, roff, FTOT, tuple(TL.ravel()),
                 tuple(int(x) for s in fblocks for x in s))

    in_maps = []
    for c in range(NC):
        d = dict(W)
        d["xuT"] = xuT[c]; d["xmT"] = xmT[c]
        d["rowidx"] = rowidx[c]; d["dstidx"] = dstidx[c]
        d["collocal"] = collocal[c]; d["masklog"] = masklog[c]
        d["fuidx"] = fuidx[c]; d["fmidx"] = fmidx[c]
        d["fpop"] = fpop[c]
        in_maps.append(d)

    assemble = (E, edge_of_lane)
    return sched, in_maps, assemble


def _ln_relu(nc, pool, v, g_rep, b_rep, do_relu=True):
    """LayerNorm along free dim of [128,128] tile v (sbuf) -> new sbuf tile."""
    musum = pool.tile([128, 1], F32, tag="ln_musum")
    nc.vector.tensor_reduce(musum[:], v[:], axis=mybir.AxisListType.X, op=ADD)
    mu = pool.tile([128, 1], F32, tag="ln_mu")
    nc.scalar.activation(mu[:], musum[:], Copy, scale=1.0 / H)
    xc = pool.tile([128, H], F32, tag="ln_xc")
    nc.vector.tensor_scalar(out=xc[:], in0=v[:], scalar1=mu[:], scalar2=None,
                            op0=SUB)
    sq = pool.tile([128, H], F32, tag="ln_sq")
    vs = pool.tile([128, 1], F32, tag="ln_vs")
    nc.scalar.activation(sq[:], xc[:], Square, accum_out=vs[:])
    var = pool.tile([128, 1], F32, tag="ln_var")
    nc.vector.tensor_scalar(out=var[:], in0=vs[:], scalar1=1.0 / H,
                            scalar2=LN_EPS, op0=MUL, op1=ADD)
    sd = pool.tile([128, 1], F32, tag="ln_sd")
    nc.scalar.activation(sd[:], var[:], Sqrt)
    rstd = pool.tile([128, 1], F32, tag="ln_rstd")
    nc.vector.reciprocal(rstd[:], sd[:])
    xn = pool.tile([128, H], F32, tag="ln_xn")
    nc.vector.scalar_tensor_tensor(out=xn[:], in0=xc[:], scalar=rstd[:],
                                   in1=g_rep[:], op0=MUL, op1=MUL)
    y = pool.tile([128, H], F32, tag="ln_y")
    if do_relu:
        nc.vector.tensor_add(xn[:], xn[:], b_rep[:])
        nc.scalar.activation(y[:], xn[:], Relu)
    else:
        nc.vector.tensor_add(y[:], xn[:], b_rep[:])
    return y


def build(sched):
    G, UCP, HALF, TT = sched.G, sched.UCP, sched.HALF, sched.TT
    RCOLS = sched.RCOLS
    NTMAX = max(max(i["nt"]) for i in sched.sb_info)
    nc = bacc.Bacc("TRN2", target_bir_lowering=False, debug=False,
                   num_devices=NC)

    # ---------------- tensors ----------------
    inp = {}
    def add_in(name, shape, dt=F32):
        inp[name] = nc.dram_tensor(name, list(shape), dt, kind="ExternalInput")
    add_in("xuT", (sched.FU, UCP)); add_in("xmT", (sched.FM, UCP))
    add_in("rowidx", (128, RCOLS), I16); add_in("dstidx", (128, RCOLS), I16)
    add_in("collocal", (128, TT)); add_in("masklog", (128, TT))
    add_in("fuidx", (128, sched.FTOT // 16), I16)
    add_in("fmidx", (128, sched.FTOT // 16), I16)
    add_in("fpop", (1, sched.FTOT))
    for nm in ["w1uT", "b1u", "w2uT", "b2u", "gu", "beu",
               "w1mT", "b1m", "w2mT", "b2m", "gm", "bem",
               "epuT", "epmT", "epb1", "iota", "ident", "selfmask"]:
        add_in(nm, {"w1uT": (sched.FU, 128), "w1mT": (sched.FM, 128),
                    "selfmask": (128, G)}.get(nm, (128, 128)))
    for i in range(L):
        add_in(f"wc{i}T", (128, 128)); add_in(f"bc{i}", (128, 128))
        add_in(f"cg{i}", (128, 128)); add_in(f"cb{i}", (128, 128))
    for i in range(L - 1):
        add_in(f"ws{i}T", (128, 128)); add_in(f"bs{i}", (128, 128))
    add_in("ep2T", (128, 64)); add_in("ep2b", (64, 1)); add_in("ep3w", (64, 1))
    add_in("pwpb", (1, 2)); add_in("ones_c", (128, 1)); add_in("ones_r", (1, 128))

    out_dram = nc.dram_tensor("out", [1, sched.FTOT], F32, kind="ExternalOutput")

    ag_in = nc.dram_tensor("ag_in", [UCP, 256], F32)
    utable = nc.dram_tensor("utable", [NC * UCP, 256], F32, addr_space="Shared")
    mslice = nc.dram_tensor("mslice", [UCP, 256], F32)
    presk_u = nc.dram_tensor("presk_u", [UCP, H], F32)
    presk_m = nc.dram_tensor("presk_m", [UCP, H], F32)
    pp_in = nc.dram_tensor("pp_in", [UCP, 256], F32)
    pptable = nc.dram_tensor("pptable", [NC * UCP, 256], F32, addr_space="Shared")
    dbounce = nc.dram_tensor("dbounce", [1, 8], F32)
    dout_t = nc.dram_tensor("dout", [1, 8], F32, addr_space="Shared")

    from contextlib import ExitStack
    with tile.TileContext(nc) as tc, ExitStack() as ctx:
        nc.gpsimd.load_library(mlp_lib)
        cp = ctx.enter_context(tc.tile_pool(name="consts", bufs=1))
        C = {}
        for nm, t in inp.items():
            if nm in ("xuT", "xmT", "fpop"):
                continue
            C[nm] = cp.tile(list(t.shape), t.dtype, tag=f"c_{nm}", name=f"c_{nm}")
            nc.sync.dma_start(C[nm][:], t[:])

        wp = ctx.enter_context(tc.tile_pool(name="work", bufs=3))
        pp = ctx.enter_context(tc.tile_pool(name="psumw", bufs=1, space="PSUM"))

        # x tile [128n, 128h] -> x_hat/lin into SBUF staging APs; extras are
        # (wT, bias, out_sbuf_ap) matmuls from the transposed x.
        def finish_node(xt, out_xhat, lin_w, lin_b, out_lx, extras):
            if out_xhat is not None:
                sq = wp.tile([128, H], F32, tag="fn_sq")
                ss = wp.tile([128, 1], F32, tag="fn_ss")
                nc.scalar.activation(sq[:], xt[:], Square, accum_out=ss[:])
                nrm = wp.tile([128, 1], F32, tag="fn_nrm")
                nc.scalar.activation(nrm[:], ss[:], Sqrt)
                nc.vector.tensor_scalar(out=nrm[:], in0=nrm[:],
                                        scalar1=COS_EPS,
                                        scalar2=None, op0=MAX)
                q = wp.tile([128, 1], F32, tag="fn_q")
                nc.vector.reciprocal(q[:], nrm[:])
                nc.vector.tensor_scalar(out=out_xhat, in0=xt[:], scalar1=q[:],
                                        scalar2=None, op0=MUL)
            pt = pp.tile([128, H], F32, tag="ps", bufs=3)
            nc.tensor.transpose(pt[:], xt[:], C["ident"][:])
            xT = wp.tile([128, H], F32, tag="fn_xT")
            nc.vector.tensor_copy(xT[:], pt[:])
            if lin_w is not None:
                pl = pp.tile([128, H], F32, tag="ps", bufs=3)
                nc.tensor.matmul(pl[:], xT[:], lin_w[:], start=True, stop=True)
                nc.vector.tensor_add(out_lx, pl[:], lin_b[:])
            for (wT, bias, out_ap) in extras:
                pe = pp.tile([128, H], F32, tag="ps", bufs=3)
                nc.tensor.matmul(pe[:], xT[:], wT[:], start=True, stop=True)
                if bias is not None:
                    nc.vector.tensor_add(out_ap, pe[:], bias[:])
                else:
                    nc.vector.tensor_copy(out_ap, pe[:])

        # ---------------- encoders ----------------
        with tc.tile_pool(name="enc", bufs=3) as ep:
            xuT_sb = ep.tile([sched.FU, UCP], F32, tag="xuT", bufs=1)
            nc.sync.dma_start(xuT_sb[:], inp["xuT"][:])
            xmT_sb = ep.tile([sched.FM, UCP], F32, tag="xmT", bufs=1)
            nc.sync.dma_start(xmT_sb[:], inp["xmT"][:])
            for side in range(2):
                xT_in = xuT_sb if side == 0 else xmT_sb
                w1 = C["w1uT"] if side == 0 else C["w1mT"]
                b1 = C["b1u"] if side == 0 else C["b1m"]
                w2 = C["w2uT"] if side == 0 else C["w2mT"]
                b2 = C["b2u"] if side == 0 else C["b2m"]
                gr = C["gu"] if side == 0 else C["gm"]
                br = C["beu"] if side == 0 else C["bem"]
                sl = ag_in if side == 0 else mslice
                NB = 8
                for b0 in range(0, G, NB):
                    nb = min(NB, G - b0)
                    stx = ep.tile([128, NB, 256], F32, tag="enc_stx")
                    for g in range(b0, b0 + nb):
                        p1 = pp.tile([128, H], F32, tag="ps", bufs=3)
                        nc.tensor.matmul(p1[:],
                                         xT_in[:, g * 128:(g + 1) * 128],
                                         w1[:], start=True, stop=True)
                        h1 = ep.tile([128, H], F32, tag="enc_h1")
                        nc.vector.tensor_add(h1[:], p1[:], b1[:])
                        h1r = ep.tile([128, H], F32, tag="enc_h1r")
                        nc.scalar.activation(h1r[:], h1[:], Relu)
                        pt = pp.tile([128, H], F32, tag="ps", bufs=3)
                        nc.tensor.transpose(pt[:], h1r[:], C["ident"][:])
                        h1T = ep.tile([128, H], F32, tag="enc_h1T")
                        nc.vector.tensor_copy(h1T[:], pt[:])
                        p2 = pp.tile([128, H], F32, tag="ps", bufs=3)
                        nc.tensor.matmul(p2[:], h1T[:], w2[:], start=True,
                                         stop=True)
                        z = ep.tile([128, H], F32, tag="enc_z")
                        nc.vector.tensor_add(z[:], p2[:], b2[:])
                        x0 = _ln_relu(nc, ep, z, gr, br, do_relu=False)
                        t = g - b0
                        finish_node(x0, stx[:, t, 0:H], C["wc0T"], C["bc0"],
                                    stx[:, t, H:256], [])
                    nc.sync.dma_start(
                        sl[b0 * 128:(b0 + nb) * 128, :].rearrange(
                            "(t p) c -> p t c", p=128),
                        stx[:, 0:nb, :])

        # ---------------- layers ----------------
        from contextlib import ExitStack as _ES
        for l in range(L):
            lctx = _ES()
            ap = lctx.enter_context(tc.tile_pool(name=f"agg{l}", bufs=1))
            aggr = ap.tile([128, G, H], F32, tag="aggr", name=f"aggr{l}")
            dstage = ap.tile([128, sched.NSBH], F32, tag="dstage",
                             name=f"dstage{l}")
            wsu = ap.tile([128, G], F32, tag="wsu", name=f"wsu{l}")
            wsm = ap.tile([128, G], F32, tag="wsm", name=f"wsm{l}")
            recipD = ap.tile([128, 1], F32, tag="recipD", name=f"recipD{l}")
            with tc.tile_pool(name=f"edge{l}", bufs=2) as xp, \
                 tc.tile_pool(name=f"oh{l}", bufs=4) as ohp, \
                 tc.tile_pool(name=f"eps{l}", bufs=1, space="PSUM") as eps:
                nc.gpsimd.collective_compute(
                    "AllGather", mybir.AluOpType.bypass,
                    replica_groups=[list(range(NC))],
                    ins=[ag_in[:]], outs=[utable[:]])

                k_sbh = 0
                for info in sched.sb_info:
                    bufs = [None, None]
                    whs = [None, None]
                    for h2 in range(2):
                        nt = info["nt"][h2]
                        if nt == 0:
                            k_sbh += 1
                            continue
                        rb = xp.tile([128, NTMAX, 256], F32, tag="rowbuf")
                        view = utable[h2 * HALF:(h2 + 1) * HALF, :]
                        o = info["roff"][h2]
                        nc.gpsimd.dma_gather(
                            out_ap=rb[:, 0:nt, :], in_ap=view,
                            idxs_ap=C["rowidx"][:, o:o + nt * 8],
                            num_idxs=nt * 128, num_idxs_reg=nt * 128,
                            elem_size=256, single_packet=False)
                        db = xp.tile([128, NTMAX, H], F32, tag="dstbuf")
                        nc.gpsimd.dma_gather(
                            out_ap=db[:, 0:nt, :], in_ap=ag_in[:, 0:H],
                            idxs_ap=C["dstidx"][:, o:o + nt * 8],
                            num_idxs=nt * 128, num_idxs_reg=nt * 128,
                            elem_size=H, elem_step=256, single_packet=False)
                        nc.vector.tensor_mul(db[:, 0:nt, :],
                                             rb[:, 0:nt, 0:H], db[:, 0:nt, :])
                        dots = xp.tile([128, NTMAX], F32, tag="dots")
                        nc.vector.tensor_reduce(
                            dots[:, 0:nt], db[:, 0:nt, :],
                            axis=mybir.AxisListType.X, op=ADD)
                        gc0 = info["tiles"][h2][0][2]
                        nc.vector.tensor_add(dots[:, 0:nt], dots[:, 0:nt],
                                             C["masklog"][:, gc0:gc0 + nt])
                        wh = xp.tile([128, NTMAX], F32, tag="wh")
                        nc.scalar.activation(wh[:, 0:nt], dots[:, 0:nt], Exp,
                                             accum_out=dstage[:, k_sbh:k_sbh + 1])
                        bufs[h2] = rb
                        whs[h2] = wh
                        k_sbh += 1
                    for g in info["groups"]:
                        chain = []
                        for h2 in range(2):
                            for (bt, gg, gc) in info["tiles"][h2]:
                                if gg == g:
                                    chain.append((h2, bt, gc))
                        pg = eps.tile([128, H], F32, tag="pgroup", bufs=2)
                        for i, (h2, bt, gc) in enumerate(chain):
                            oh = ohp.tile([128, 128], F32, tag="oh")
                            nc.vector.tensor_scalar(
                                out=oh[:], in0=C["iota"][:],
                                scalar1=C["collocal"][:, gc:gc + 1],
                                scalar2=whs[h2][:, bt:bt + 1],
                                op0=ISEQ, op1=MUL)
                            nc.tensor.matmul(
                                pg[:], oh[:], bufs[h2][:, bt, H:256],
                                start=(i == 0), stop=(i == len(chain) - 1))
                        nc.vector.tensor_copy(aggr[:, g, :], pg[:])

            NB = 8
            with tc.tile_pool(name=f"node{l}", bufs=2) as npo, \
                 tc.tile_pool(name=f"nps{l}", bufs=1, space="PSUM") as nps:
                # self sims (batched row loads)
                for side in range(2):
                    sl = ag_in if side == 0 else mslice
                    ws = wsu if side == 0 else wsm
                    for b0 in range(0, G, NB):
                        nb = min(NB, G - b0)
                        srows = npo.tile([128, NB, 256], F32, tag="ss_rows")
                        nc.sync.dma_start(
                            srows[:, 0:nb, :],
                            sl[b0 * 128:(b0 + nb) * 128, :].rearrange(
                                "(t p) c -> p t c", p=128))
                        for g in range(b0, b0 + nb):
                            t = g - b0
                            sq = npo.tile([128, H], F32, tag="self_sq")
                            ss = npo.tile([128, 1], F32, tag="self_ss")
                            nc.scalar.activation(sq[:], srows[:, t, 0:H],
                                                 Square, accum_out=ss[:])
                            nc.scalar.activation(ws[:, g:g + 1], ss[:], Exp,
                                                 bias=C["selfmask"][:, g:g + 1])

                # D total + allreduce
                d1 = npo.tile([128, 1], F32, tag="d1")
                nc.vector.tensor_reduce(d1[:], dstage[:],
                                        axis=mybir.AxisListType.X, op=ADD)
                d2 = npo.tile([128, 1], F32, tag="d2")
                nc.vector.tensor_reduce(d2[:], wsu[:],
                                        axis=mybir.AxisListType.X, op=ADD)
                d3 = npo.tile([128, 1], F32, tag="d3")
                nc.vector.tensor_reduce(d3[:], wsm[:],
                                        axis=mybir.AxisListType.X, op=ADD)
                nc.vector.tensor_add(d1[:], d1[:], d2[:])
                nc.vector.tensor_add(d1[:], d1[:], d3[:])
                pd = nps.tile([1, 1], F32, tag="pmisc", bufs=1)
                nc.tensor.matmul(pd[:], d1[:], C["ones_c"][:], start=True,
                                 stop=True)
                dsb = npo.tile([1, 1], F32, tag="dsb")
                nc.vector.tensor_copy(dsb[:], pd[:])
                nc.sync.dma_start(dbounce[0:1, 0:1], dsb[:])
                nc.gpsimd.collective_compute(
                    "AllReduce", ADD, replica_groups=[list(range(NC))],
                    ins=[dbounce[:]], outs=[dout_t[:]])
                dval = npo.tile([1, 1], F32, tag="dval")
                nc.sync.dma_start(dval[:], dout_t[0:1, 0:1])
                pb = nps.tile([128, 1], F32, tag="pmisc", bufs=1)
                nc.tensor.matmul(pb[:], C["ones_r"][:], dval[:], start=True,
                                 stop=True)
                nc.vector.reciprocal(recipD[:], pb[:])

                # ---------------- node update (batched I/O) ----------------
                last = (l == L - 1)
                for side in range(2):
                    sl = ag_in if side == 0 else mslice
                    psk = presk_u if side == 0 else presk_m
                    ws = wsu if side == 0 else wsm
                    for b0 in range(0, G, NB):
                        nb = min(NB, G - b0)
                        rows = npo.tile([128, NB, 256], F32, tag="nd_rows")
                        nc.sync.dma_start(
                            rows[:, 0:nb, :],
                            sl[b0 * 128:(b0 + nb) * 128, :].rearrange(
                                "(t p) c -> p t c", p=128))
                        if l > 0:
                            pkb = npo.tile([128, NB, H], F32, tag="nd_pkb")
                            nc.sync.dma_start(
                                pkb[:, 0:nb, :],
                                psk[b0 * 128:(b0 + nb) * 128, :].rearrange(
                                    "(t p) c -> p t c", p=128))
                        stx = npo.tile([128, NB, 256], F32, tag="nd_stx")
                        stp = npo.tile([128, NB, H], F32, tag="nd_stp")
                        for g in range(b0, b0 + nb):
                            t = g - b0
                            lx = rows[:, t, H:256]
                            t1 = npo.tile([128, H], F32, tag="nd_t1")
                            if side == 0:
                                nc.vector.scalar_tensor_tensor(
                                    out=t1[:], in0=lx, scalar=ws[:, g:g + 1],
                                    in1=aggr[:, g, :], op0=MUL, op1=ADD)
                            else:
                                nc.vector.tensor_scalar(
                                    out=t1[:], in0=lx, scalar1=ws[:, g:g + 1],
                                    scalar2=None, op0=MUL)
                            nc.vector.tensor_scalar(out=t1[:], in0=t1[:],
                                                    scalar1=recipD[:],
                                                    scalar2=None, op0=MUL)
                            v = npo.tile([128, H], F32, tag="nd_v")
                            nc.vector.tensor_add(v[:], t1[:], lx)
                            xr = _ln_relu(nc, npo, v, C[f"cg{l}"],
                                          C[f"cb{l}"], do_relu=True)
                            if l > 0:
                                nc.vector.tensor_add(xr[:], xr[:],
                                                     pkb[:, t, :])
                            if not last:
                                finish_node(xr, stx[:, t, 0:H],
                                            C[f"wc{l + 1}T"], C[f"bc{l + 1}"],
                                            stx[:, t, H:256],
                                            [(C[f"ws{l}T"], C[f"bs{l}"],
                                              stp[:, t, :])])
                            else:
                                wext = (C["epuT"] if side == 0 else C["epmT"])
                                bext = (C["epb1"] if side == 0 else None)
                                finish_node(xr, None, None, None, None,
                                            [(wext, bext, stp[:, t, :])])
                        if not last:
                            nc.sync.dma_start(
                                sl[b0 * 128:(b0 + nb) * 128, :].rearrange(
                                    "(t p) c -> p t c", p=128),
                                stx[:, 0:nb, :])
                            nc.sync.dma_start(
                                psk[b0 * 128:(b0 + nb) * 128, :].rearrange(
                                    "(t p) c -> p t c", p=128),
                                stp[:, 0:nb, :])
                        else:
                            c0 = 0 if side == 0 else H
                            nc.sync.dma_start(
                                pp_in[b0 * 128:(b0 + nb) * 128,
                                      c0:c0 + H].rearrange(
                                    "(t p) c -> p t c", p=128),
                                stp[:, 0:nb, :])
            lctx.close()

        # ---------------- final MLP ----------------
        with tc.tile_pool(name="fin", bufs=2) as fp, \
             tc.tile_pool(name="fps", bufs=1, space="PSUM") as fps, \
             tc.tile_pool(name="fout", bufs=2) as fop:
            nc.gpsimd.collective_compute(
                "AllGather", mybir.AluOpType.bypass,
                replica_groups=[list(range(NC))],
                ins=[pp_in[:]], outs=[pptable[:]])
            CHUNK = 4096
            ost = None
            ost_base = 0
            for (s, bt, lo) in sched.fblocks:
                uh = s
                pu = fp.tile([128, 16, H], F32, tag="f_pu")
                nc.gpsimd.dma_gather(
                    out_ap=pu[:, 0:bt, :],
                    in_ap=pptable[uh * HALF:(uh + 1) * HALF, 0:H],
                    idxs_ap=C["fuidx"][:, lo // 16:(lo + bt * 128) // 16],
                    num_idxs=bt * 128, num_idxs_reg=bt * 128,
                    elem_size=H, elem_step=256, single_packet=False)
                pm = fp.tile([128, 16, H], F32, tag="f_pm")
                nc.gpsimd.dma_gather(
                    out_ap=pm[:, 0:bt, :],
                    in_ap=pp_in[:, H:256],
                    idxs_ap=C["fmidx"][:, lo // 16:(lo + bt * 128) // 16],
                    num_idxs=bt * 128, num_idxs_reg=bt * 128,
                    elem_size=H, elem_step=256, single_packet=False)
                h1 = fp.tile([128, 16, H], F32, tag="f_h1")
                nc.vector.tensor_add(h1[:, 0:bt, :], pu[:, 0:bt, :],
                                     pm[:, 0:bt, :])
                h1r = fp.tile([128, 16, H], F32, tag="f_h1r")
                nc.scalar.activation(h1r[:, 0:bt, :], h1[:, 0:bt, :], Relu)
                for t in range(bt):
                    ptr = fps.tile([128, H], F32, tag="f_tr", bufs=2)
                    nc.tensor.transpose(ptr[:], h1r[:, t, :], C["ident"][:])
                    h1T = fp.tile([128, H], F32, tag="f_h1T")
                    nc.vector.tensor_copy(h1T[:], ptr[:])
                    p2 = fps.tile([64, 128], F32, tag="f_p2", bufs=2)
                    nc.tensor.matmul(p2[:], C["ep2T"][:], h1T[:], start=True,
                                     stop=True)
                    h2r = fp.tile([64, 128], F32, tag="f_h2r")
                    nc.scalar.activation(h2r[:], p2[:], Relu,
                                         bias=C["ep2b"][:])
                    p3 = fps.tile([1, 128], F32, tag="f_p3", bufs=1)
                    nc.tensor.matmul(p3[:], C["ep3w"][:], h2r[:], start=True,
                                     stop=True)
                    lane0 = lo + t * 128
                    if ost is None or lane0 - ost_base >= CHUNK:
                        if ost is not None:
                            nc.sync.dma_start(
                                out_dram[0:1, ost_base:ost_base + CHUNK],
                                ost[:])
                        ost = fop.tile([1, CHUNK], F32, tag="f_ost", bufs=1)
                        ost_base = lane0
                        popc = fop.tile([1, CHUNK], F32, tag="f_popc", bufs=1)
                        nc.sync.dma_start(
                            popc[:], inp["fpop"][0:1, ost_base:ost_base + CHUNK])
                        nc.vector.tensor_scalar(
                            out=ost[:], in0=popc[:],
                            scalar1=C["pwpb"][0:1, 0:1],
                            scalar2=C["pwpb"][0:1, 1:2], op0=MUL, op1=ADD)
                    co = lane0 - ost_base
                    nc.vector.tensor_add(ost[0:1, co:co + 128],
                                         ost[0:1, co:co + 128], p3[:])
            if ost is not None:
                nc.sync.dma_start(out_dram[0:1, ost_base:ost_base + CHUNK],
                                  ost[:])

    nc.compile()
    return nc


_CACHE = {}
LAST_RESULT = None


def kernel(x_user, x_movie, edge_index, movie_popularity, params):
    x_user = np.asarray(x_user)
    x_movie = np.asarray(x_movie)
    edge_index = np.asarray(edge_index)
    movie_popularity = np.asarray(movie_popularity)
    sched, in_maps, (E, edge_of_lane) = _prep(
        x_user, x_movie, edge_index, movie_popularity, params)
    nc = _CACHE.get(sched.key)
    if nc is None:
        nc = build(sched)
        _CACHE[sched.key] = nc
    res = run_bass_kernel_spmd(nc, in_maps, core_ids=list(range(NC)))
    global LAST_RESULT
    LAST_RESULT = res
    y = np.zeros((E, 1), np.float32)
    for c in range(NC):
        lanes = edge_of_lane[c]
        m = lanes >= 0
        y[lanes[m], 0] = res.results[c]["out"][0][m]
    return y


# revision 15
# speedup vs baseline: 12.7636x; 12.7335x over previous
"""Trainium2 Bass kernel for EnhancedMPGNN (attention GNN + edge MLP).

Strategy (edge-parallel by destination column):
  - 500k random edges sharded across 8 cores by col ownership (6250 cols/core).
    Self-loop edges handled analytically node-parallel (no gathers).
  - Node tables replicated per layer via AllGather of [x_hat | lin(x)] slices.
  - Per-edge gathers via dma_gather (custom SWDGE instruction), cosine sim from
    pre-normalized x_hat, global softmax denominator via scalar AllReduce.
  - Scatter-add realized as one-hot matmuls accumulating in PSUM per 128-col
    group; aggregate stays in SBUF (no scatter instruction at all).
  - Final edge MLP: ep1 split into per-user/per-movie node projections
    (linearity), per-edge gathers + small matmuls.
"""
import sys
import numpy as np

sys.path.insert(0, "/opt/trn_rl_repo")

import concourse.bass as bass
import concourse.bacc as bacc
import concourse.tile as tile
import concourse.mybir as mybir
from concourse.bass_utils import run_bass_kernel_spmd
from concourse.library_config import mlp as mlp_lib

F32 = mybir.dt.float32
BF16 = mybir.dt.bfloat16
I16 = mybir.dt.int16
NC = 8
H = 128
L = 3
LN_EPS = 1e-5
COS_EPS = 1e-8
NEG = -30000.0  # exp(x+NEG) == 0.0 in f32
Relu = mybir.ActivationFunctionType.Relu
Exp = mybir.ActivationFunctionType.Exp
Sqrt = mybir.ActivationFunctionType.Sqrt
Square = mybir.ActivationFunctionType.Square
Copy = mybir.ActivationFunctionType.Copy
ADD = mybir.AluOpType.add
SUB = mybir.AluOpType.subtract
MUL = mybir.AluOpType.mult
MAX = mybir.AluOpType.max
ISEQ = mybir.AluOpType.is_equal


def _wrap_idx(seg):
    """int16 list (len%16==0) -> [16, n/16] wrapped block."""
    n = len(seg)
    return np.asarray(seg, np.int16).reshape(n // 16, 16).T


class Sched:
    pass


def _prep(x_user, x_movie, edge_index, movie_popularity, params):
    """Host-side sharding/scheduling. Returns (sched, in_maps, assemble)."""
    NUSER = x_user.shape[0]
    NMOV = x_movie.shape[0]
    E = edge_index.shape[1]
    row = np.asarray(edge_index[0], np.int64)
    col = np.asarray(edge_index[1], np.int64)
    UC = NUSER // NC              # owned user cols per core
    G = (UC + 127) // 128         # 128-col groups per core (= node tiles/side)
    UCP = G * 128                 # padded rows per core slice
    HALF = (NC // 2) * UCP        # row-gather table half size
    assert HALF < 32768 and UCP < 32768

    def r_of(n):                  # user node -> table row
        return UCP * (n // UC) + n % UC

    rtab = r_of(row)
    lo_edge = rtab < HALF
    core_of = col // UC

    # ---- edge phase schedule: per (core, group, half) edge lists ----
    per = [[[None, None] for _ in range(G)] for _ in range(NC)]
    for c in range(NC):
        m = np.nonzero(core_of == c)[0]
        gg = np.minimum((col[m] - c * UC) // 128, G - 1)
        lo = lo_edge[m]
        for g in range(G):
            sel = m[(gg == g)]
            losel = lo[(gg == g)]
            per[c][g][0] = sel[losel]
            per[c][g][1] = sel[~losel]
    TL = np.zeros((G, 2), np.int64)  # tiles per (group, half), shared
    for g in range(G):
        for h in range(2):
            mx = max(len(per[c][g][h]) for c in range(NC))
            TL[g, h] = max(1, (mx + 127) // 128)

    # superblocks of 2 groups
    SBS = [list(range(i, min(i + 2, G))) for i in range(0, G, 2)]
    sb_info = []
    roff = 0      # idx column offset (16 idx per column)
    gcol = 0      # global tile column (collocal / masklog)
    for groups in SBS:
        info = {"groups": groups, "nt": [0, 0], "roff": [0, 0],
                "tiles": [[], []]}
        for h in range(2):
            nt = int(sum(TL[g, h] for g in groups))
            info["nt"][h] = nt
            info["roff"][h] = roff
            bt = 0
            for g in groups:
                for _ in range(int(TL[g, h])):
                    info["tiles"][h].append((bt, g, gcol))
                    bt += 1
                    gcol += 1
            roff += nt * 8
        sb_info.append(info)
    TT = gcol
    RTOT = roff * 16            # total lanes

    # per-core lane arrays for edge phase
    rowidx = np.zeros((NC, 128, roff), np.int16)
    dstidx = np.zeros((NC, 128, roff), np.int16)
    collocal = np.zeros((NC, 128, TT), np.float32)
    masklog = np.full((NC, 128, TT), NEG, np.float32)
    for c in range(NC):
        for info in sb_info:
            for h in range(2):
                nt = info["nt"][h]
                if nt == 0:
                    continue
                lanes_r = np.zeros(nt * 128, np.int16)
                lanes_d = np.zeros(nt * 128, np.int16)
                pos = 0
                for g in info["groups"]:
                    tg = int(TL[g, h])
                    e = per[c][g][h]
                    k = len(e)
                    sl = slice(pos * 128, pos * 128 + k)
                    lanes_r[sl] = (rtab[e] - h * HALF).astype(np.int16)
                    lanes_d[sl] = (col[e] - c * UC).astype(np.int16)
                    # per-tile metadata columns
                    for t in range(tg):
                        gc = info["tiles"][h][pos + t][2]
                        lo_l = t * 128
                        hi_l = min(k, lo_l + 128)
                        nval = max(0, hi_l - lo_l)
                        if nval > 0:
                            cl = col[e[lo_l:hi_l]] - c * UC - 128 * g
                            collocal[c, :nval, gc] = cl.astype(np.float32)
                            masklog[c, :nval, gc] = 0.0
                    pos += tg
                o = info["roff"][h]
                rowidx[c, :, o:o + nt * 8] = np.tile(
                    _wrap_idx(lanes_r), (8, 1)).reshape(128, nt * 8)
                dstidx[c, :, o:o + nt * 8] = np.tile(
                    _wrap_idx(lanes_d), (8, 1)).reshape(128, nt * 8)

    # ---- self masks ----
    flat = np.arange(UCP)
    sm = np.where(flat < UC, 0.0, NEG).astype(np.float32)
    selfmask = sm.reshape(G, 128).T.copy()  # [128, G]

    # ---- final MLP schedule: 4 segments by (uhalf, mhalf) ----
    # pm is gathered from the core's PRIVATE pp_in (its own movie nodes),
    # so only the pu table half splits lanes: 2 segments by uhalf.
    NSEG = 2
    segid = (rtab >= HALF).astype(np.int64)
    fseg = [[None] * NSEG for _ in range(NC)]
    FS = np.zeros(NSEG, np.int64)
    for c in range(NC):
        m = np.nonzero(core_of == c)[0]
        for s in range(NSEG):
            fseg[c][s] = m[segid[m] == s]
            FS[s] = max(FS[s], (len(fseg[c][s]) + 127) // 128)
    FS = np.maximum(FS, 1)
    # blocks of up to 16 tiles per segment
    fblocks = []   # (seg, ntiles, lane_off)
    lane_off = 0
    for s in range(NSEG):
        t = 0
        while t < FS[s]:
            bt = int(min(16, FS[s] - t))
            fblocks.append((s, bt, lane_off))
            lane_off += bt * 128
            t += bt
    FTOT = ((lane_off + 4095) // 4096) * 4096
    fuidx = np.zeros((NC, 128, FTOT // 16), np.int16)
    fmidx = np.zeros((NC, 128, FTOT // 16), np.int16)
    fpop = np.zeros((NC, 1, FTOT), np.float32)
    edge_of_lane = np.full((NC, FTOT), -1, np.int64)
    fill_pos = np.zeros((NC, 4), np.int64)
    seg_base = {}
    off = 0
    for s in range(NSEG):
        seg_base[s] = off
        off += int(FS[s]) * 128
    pop = np.asarray(movie_popularity, np.float32)
    for c in range(NC):
        lanes_u = np.zeros(FTOT, np.int16)
        lanes_m = np.zeros(FTOT, np.int16)
        for s in range(NSEG):
            e = fseg[c][s]
            b = seg_base[s]
            lanes_u[b:b + len(e)] = (rtab[e] - s * HALF).astype(np.int16)
            lanes_m[b:b + len(e)] = (col[e] - c * UC).astype(np.int16)
            fpop[c, 0, b:b + len(e)] = pop[e]
            edge_of_lane[c, b:b + len(e)] = e
        # wrap per block
        for s, bt, lo in fblocks:
            n = bt * 128
            fuidx[c, :, lo // 16:(lo + n) // 16] = np.tile(
                _wrap_idx(lanes_u[lo:lo + n]), (8, 1)).reshape(128, n // 16)
            fmidx[c, :, lo // 16:(lo + n) // 16] = np.tile(
                _wrap_idx(lanes_m[lo:lo + n]), (8, 1)).reshape(128, n // 16)

    # ---- encoder inputs (transposed, padded, per core) ----
    FU = x_user.shape[1]
    FM = x_movie.shape[1]
    xuT = np.zeros((NC, FU, UCP), np.float32)
    xmT = np.zeros((NC, FM, UCP), np.float32)
    for c in range(NC):
        xuT[c, :, :UC] = np.asarray(x_user[c * UC:(c + 1) * UC], np.float32).T
        xmT[c, :, :UC] = np.asarray(x_movie[c * UC:(c + 1) * UC], np.float32).T

    # ---- weights (host-transposed / replicated) ----
    P = params
    rep = lambda v: np.tile(np.asarray(v, np.float32).reshape(1, -1), (128, 1))
    W = {}
    ue, me = P["user_enc"], P["movie_enc"]
    W["w1uT"] = np.asarray(ue["l1"]["w"], np.float32).T.copy()   # [32,128]
    W["b1u"] = rep(ue["l1"]["b"])
    W["w2uT"] = np.asarray(ue["l2"]["w"], np.float32).T.copy()
    W["b2u"] = rep(ue["l2"]["b"])
    W["gu"] = rep(ue["g"]); W["beu"] = rep(ue["beta"])
    W["w1mT"] = np.asarray(me["l1"]["w"], np.float32).T.copy()   # [64,128]
    W["b1m"] = rep(me["l1"]["b"])
    W["w2mT"] = np.asarray(me["l2"]["w"], np.float32).T.copy()
    W["b2m"] = rep(me["l2"]["b"])
    W["gm"] = rep(me["g"]); W["bem"] = rep(me["beta"])
    for i in range(L):
        cv = P["conv"][i]
        W[f"wc{i}T"] = np.asarray(cv["lin"]["w"], np.float32).T.copy()
        W[f"bc{i}"] = rep(cv["lin"]["b"])
        W[f"cg{i}"] = rep(cv["g"]); W[f"cb{i}"] = rep(cv["beta"])
    for i in range(L - 1):
        sk = P["skip"][i]
        W[f"ws{i}T"] = np.asarray(sk["w"], np.float32).T.copy()
        W[f"bs{i}"] = rep(sk["b"])
    ep1w = np.asarray(P["ep1"]["w"], np.float32)   # [128, 256]
    W["epuT"] = ep1w[:, :H].T.copy()
    W["epmT"] = ep1w[:, H:].T.copy()
    W["epb1"] = rep(P["ep1"]["b"])
    W["ep2T"] = np.asarray(P["ep2"]["w"], np.float32).T.copy()   # [128, 64]
    W["ep2b"] = np.asarray(P["ep2"]["b"], np.float32).reshape(64, 1).copy()
    W["ep3w"] = np.asarray(P["ep3"]["w"], np.float32).reshape(64, 1).copy()
    ep3b = float(np.asarray(P["ep3"]["b"]).reshape(()))
    popw = float(np.asarray(P["pop"]["w"]).reshape(()))
    popb = float(np.asarray(P["pop"]["b"]).reshape(())) + ep3b
    W["pwpb"] = np.array([[popw, popb]], np.float32)
    W["iota"] = np.tile(np.arange(128, dtype=np.float32), (128, 1)).copy()
    W["ident"] = np.eye(128, dtype=np.float32)
    W["ones_c"] = np.ones((128, 1), np.float32)
    W["ones_r"] = np.ones((1, 128), np.float32)
    W["selfmask"] = selfmask

    sched = Sched()
    sched.UC, sched.G, sched.UCP, sched.HALF = UC, G, UCP, HALF
    sched.TL, sched.sb_info, sched.TT, sched.RCOLS = TL, sb_info, TT, roff
    sched.FS, sched.fblocks, sched.FTOT = FS, fblocks, FTOT
    sched.FU, sched.FM = FU, FM
    sched.NSBH = 2 * len(sb_info)
    sched.key = (UC, G, TT, roff, FTOT, tuple(TL.ravel()),
                 tuple(int(x) for s in fblocks for x in s))

    in_maps = []
    for c in range(NC):
        d = dict(W)
        d["xuT"] = xuT[c]; d["xmT"] = xmT[c]
        d["rowidx"] = rowidx[c]; d["dstidx"] = dstidx[c]
        d["collocal"] = collocal[c]; d["masklog"] = masklog[c]
        d["fuidx"] = fuidx[c]; d["fmidx"] = fmidx[c]
        d["fpop"] = fpop[c]
        in_maps.append(d)

    assemble = (E, edge_of_lane)
    return sched, in_maps, assemble


def _ln_relu(nc, pool, v, g_rep, b_rep, do_relu=True):
    """LayerNorm along free dim of [128,128] tile v (sbuf) -> new sbuf tile."""
    musum = pool.tile([128, 1], F32, tag="ln_musum")
    nc.vector.tensor_reduce(musum[:], v[:], axis=mybir.AxisListType.X, op=ADD)
    mu = pool.tile([128, 1], F32, tag="ln_mu")
    nc.scalar.activation(mu[:], musum[:], Copy, scale=1.0 / H)
    xc = pool.tile([128, H], F32, tag="ln_xc")
    nc.vector.tensor_scalar(out=xc[:], in0=v[:], scalar1=mu[:], scalar2=None,
                            op0=SUB)
    sq = pool.tile([128, H], F32, tag="ln_sq")
    vs = pool.tile([128, 1], F32, tag="ln_vs")
    nc.scalar.activation(sq[:], xc[:], Square, accum_out=vs[:])
    var = pool.tile([128, 1], F32, tag="ln_var")
    nc.vector.tensor_scalar(out=var[:], in0=vs[:], scalar1=1.0 / H,
                            scalar2=LN_EPS, op0=MUL, op1=ADD)
    sd = pool.tile([128, 1], F32, tag="ln_sd")
    nc.scalar.activation(sd[:], var[:], Sqrt)
    rstd = pool.tile([128, 1], F32, tag="ln_rstd")
    nc.vector.reciprocal(rstd[:], sd[:])
    xn = pool.tile([128, H], F32, tag="ln_xn")
    nc.vector.scalar_tensor_tensor(out=xn[:], in0=xc[:], scalar=rstd[:],
                                   in1=g_rep[:], op0=MUL, op1=MUL)
    y = pool.tile([128, H], F32, tag="ln_y")
    if do_relu:
        nc.vector.tensor_add(xn[:], xn[:], b_rep[:])
        nc.scalar.activation(y[:], xn[:], Relu)
    else:
        nc.vector.tensor_add(y[:], xn[:], b_rep[:])
    return y


def build(sched):
    G, UCP, HALF, TT = sched.G, sched.UCP, sched.HALF, sched.TT
    RCOLS = sched.RCOLS
    NTMAX = max(max(i["nt"]) for i in sched.sb_info)
    nc = bacc.Bacc("TRN2", target_bir_lowering=False, debug=False,
                   num_devices=NC)

    # ---------------- tensors ----------------
    inp = {}
    def add_in(name, shape, dt=F32):
        inp[name] = nc.dram_tensor(name, list(shape), dt, kind="ExternalInput")
    add_in("xuT", (sched.FU, UCP)); add_in("xmT", (sched.FM, UCP))
    add_in("rowidx", (128, RCOLS), I16); add_in("dstidx", (128, RCOLS), I16)
    add_in("collocal", (128, TT)); add_in("masklog", (128, TT))
    add_in("fuidx", (128, sched.FTOT // 16), I16)
    add_in("fmidx", (128, sched.FTOT // 16), I16)
    add_in("fpop", (1, sched.FTOT))
    for nm in ["w1uT", "b1u", "w2uT", "b2u", "gu", "beu",
               "w1mT", "b1m", "w2mT", "b2m", "gm", "bem",
               "epuT", "epmT", "epb1", "iota", "ident", "selfmask"]:
        add_in(nm, {"w1uT": (sched.FU, 128), "w1mT": (sched.FM, 128),
                    "selfmask": (128, G)}.get(nm, (128, 128)))
    for i in range(L):
        add_in(f"wc{i}T", (128, 128)); add_in(f"bc{i}", (128, 128))
        add_in(f"cg{i}", (128, 128)); add_in(f"cb{i}", (128, 128))
    for i in range(L - 1):
        add_in(f"ws{i}T", (128, 128)); add_in(f"bs{i}", (128, 128))
    add_in("ep2T", (128, 64)); add_in("ep2b", (64, 1)); add_in("ep3w", (64, 1))
    add_in("pwpb", (1, 2)); add_in("ones_c", (128, 1)); add_in("ones_r", (1, 128))

    out_dram = nc.dram_tensor("out", [1, sched.FTOT], F32, kind="ExternalOutput")

    ag_in = nc.dram_tensor("ag_in", [UCP, 256], F32)
    utable = nc.dram_tensor("utable", [NC * UCP, 256], F32, addr_space="Shared")
    mslice = nc.dram_tensor("mslice", [UCP, 256], F32)
    presk_u = nc.dram_tensor("presk_u", [UCP, H], F32)
    presk_m = nc.dram_tensor("presk_m", [UCP, H], F32)
    pp_in = nc.dram_tensor("pp_in", [UCP, 256], F32)
    pptable = nc.dram_tensor("pptable", [NC * UCP, 256], F32, addr_space="Shared")
    dbounce = nc.dram_tensor("dbounce", [1, 8], F32)
    dout_t = nc.dram_tensor("dout", [1, 8], F32, addr_space="Shared")

    from contextlib import ExitStack
    with tile.TileContext(nc) as tc, ExitStack() as ctx:
        nc.gpsimd.load_library(mlp_lib)
        cp = ctx.enter_context(tc.tile_pool(name="consts", bufs=1))
        C = {}
        for nm, t in inp.items():
            if nm in ("xuT", "xmT", "fpop"):
                continue
            C[nm] = cp.tile(list(t.shape), t.dtype, tag=f"c_{nm}", name=f"c_{nm}")
            nc.sync.dma_start(C[nm][:], t[:])

        wp = ctx.enter_context(tc.tile_pool(name="work", bufs=3))
        pp = ctx.enter_context(tc.tile_pool(name="psumw", bufs=1, space="PSUM"))

        # x tile [128n, 128h] -> x_hat/lin into SBUF staging APs; extras are
        # (wT, bias, out_sbuf_ap) matmuls from the transposed x.
        def finish_node(xt, out_xhat, lin_w, lin_b, out_lx, extras):
            if out_xhat is not None:
                sq = wp.tile([128, H], F32, tag="fn_sq")
                ss = wp.tile([128, 1], F32, tag="fn_ss")
                nc.scalar.activation(sq[:], xt[:], Square, accum_out=ss[:])
                nrm = wp.tile([128, 1], F32, tag="fn_nrm")
                nc.scalar.activation(nrm[:], ss[:], Sqrt)
                nc.vector.tensor_scalar(out=nrm[:], in0=nrm[:],
                                        scalar1=COS_EPS,
                                        scalar2=None, op0=MAX)
                q = wp.tile([128, 1], F32, tag="fn_q")
                nc.vector.reciprocal(q[:], nrm[:])
                nc.vector.tensor_scalar(out=out_xhat, in0=xt[:], scalar1=q[:],
                                        scalar2=None, op0=MUL)
            pt = pp.tile([128, H], F32, tag="ps", bufs=3)
            nc.tensor.transpose(pt[:], xt[:], C["ident"][:])
            xT = wp.tile([128, H], F32, tag="fn_xT")
            nc.scalar.copy(xT[:], pt[:])
            if lin_w is not None:
                pl = pp.tile([128, H], F32, tag="ps", bufs=3)
                nc.tensor.matmul(pl[:], xT[:], lin_w[:], start=True, stop=True)
                nc.vector.tensor_add(out_lx, pl[:], lin_b[:])
            for (wT, bias, out_ap) in extras:
                pe = pp.tile([128, H], F32, tag="ps", bufs=3)
                nc.tensor.matmul(pe[:], xT[:], wT[:], start=True, stop=True)
                if bias is not None:
                    nc.vector.tensor_add(out_ap, pe[:], bias[:])
                else:
                    nc.vector.tensor_copy(out_ap, pe[:])

        # ---------------- encoders ----------------
        with tc.tile_pool(name="enc", bufs=3) as ep:
            xuT_sb = ep.tile([sched.FU, UCP], F32, tag="xuT", bufs=1)
            nc.sync.dma_start(xuT_sb[:], inp["xuT"][:])
            xmT_sb = ep.tile([sched.FM, UCP], F32, tag="xmT", bufs=1)
            nc.sync.dma_start(xmT_sb[:], inp["xmT"][:])
            for side in range(2):
                xT_in = xuT_sb if side == 0 else xmT_sb
                w1 = C["w1uT"] if side == 0 else C["w1mT"]
                b1 = C["b1u"] if side == 0 else C["b1m"]
                w2 = C["w2uT"] if side == 0 else C["w2mT"]
                b2 = C["b2u"] if side == 0 else C["b2m"]
                gr = C["gu"] if side == 0 else C["gm"]
                br = C["beu"] if side == 0 else C["bem"]
                sl = ag_in if side == 0 else mslice
                NB = 8
                for b0 in range(0, G, NB):
                    nb = min(NB, G - b0)
                    stx = ep.tile([128, NB, 256], F32, tag="enc_stx")
                    for g in range(b0, b0 + nb):
                        p1 = pp.tile([128, H], F32, tag="ps", bufs=3)
                        nc.tensor.matmul(p1[:],
                                         xT_in[:, g * 128:(g + 1) * 128],
                                         w1[:], start=True, stop=True)
                        h1 = ep.tile([128, H], F32, tag="enc_h1")
                        nc.vector.tensor_add(h1[:], p1[:], b1[:])
                        h1r = ep.tile([128, H], F32, tag="enc_h1r")
                        nc.scalar.activation(h1r[:], h1[:], Relu)
                        pt = pp.tile([128, H], F32, tag="ps", bufs=3)
                        nc.tensor.transpose(pt[:], h1r[:], C["ident"][:])
                        h1T = ep.tile([128, H], F32, tag="enc_h1T")
                        nc.vector.tensor_copy(h1T[:], pt[:])
                        p2 = pp.tile([128, H], F32, tag="ps", bufs=3)
                        nc.tensor.matmul(p2[:], h1T[:], w2[:], start=True,
                                         stop=True)
                        z = ep.tile([128, H], F32, tag="enc_z")
                        nc.vector.tensor_add(z[:], p2[:], b2[:])
                        x0 = _ln_relu(nc, ep, z, gr, br, do_relu=False)
                        t = g - b0
                        finish_node(x0, stx[:, t, 0:H], C["wc0T"], C["bc0"],
                                    stx[:, t, H:256], [])
                    nc.sync.dma_start(
                        sl[b0 * 128:(b0 + nb) * 128, :].rearrange(
                            "(t p) c -> p t c", p=128),
                        stx[:, 0:nb, :])

        # ---------------- layers ----------------
        from contextlib import ExitStack as _ES
        for l in range(L):
            lctx = _ES()
            ap = lctx.enter_context(tc.tile_pool(name=f"agg{l}", bufs=1))
            aggr = ap.tile([128, G, H], F32, tag="aggr", name=f"aggr{l}")
            dstage = ap.tile([128, sched.NSBH], F32, tag="dstage",
                             name=f"dstage{l}")
            wsu = ap.tile([128, G], F32, tag="wsu", name=f"wsu{l}")
            wsm = ap.tile([128, G], F32, tag="wsm", name=f"wsm{l}")
            recipD = ap.tile([128, 1], F32, tag="recipD", name=f"recipD{l}")
            with tc.tile_pool(name=f"edge{l}", bufs=2) as xp, \
                 tc.tile_pool(name=f"oh{l}", bufs=4) as ohp, \
                 tc.tile_pool(name=f"eps{l}", bufs=1, space="PSUM") as eps:
                nc.gpsimd.collective_compute(
                    "AllGather", mybir.AluOpType.bypass,
                    replica_groups=[list(range(NC))],
                    ins=[ag_in[:]], outs=[utable[:]])

                k_sbh = 0
                for info in sched.sb_info:
                    bufs = [None, None]
                    whs = [None, None]
                    for h2 in range(2):
                        nt = info["nt"][h2]
                        if nt == 0:
                            k_sbh += 1
                            continue
                        rb = xp.tile([128, NTMAX, 256], F32, tag="rowbuf")
                        view = utable[h2 * HALF:(h2 + 1) * HALF, :]
                        o = info["roff"][h2]
                        nc.gpsimd.dma_gather(
                            out_ap=rb[:, 0:nt, :], in_ap=view,
                            idxs_ap=C["rowidx"][:, o:o + nt * 8],
                            num_idxs=nt * 128, num_idxs_reg=nt * 128,
                            elem_size=256, single_packet=False)
                        db = xp.tile([128, NTMAX, H], F32, tag="dstbuf")
                        nc.gpsimd.dma_gather(
                            out_ap=db[:, 0:nt, :], in_ap=ag_in[:, 0:H],
                            idxs_ap=C["dstidx"][:, o:o + nt * 8],
                            num_idxs=nt * 128, num_idxs_reg=nt * 128,
                            elem_size=H, elem_step=256, single_packet=False)
                        nc.vector.tensor_mul(db[:, 0:nt, :],
                                             rb[:, 0:nt, 0:H], db[:, 0:nt, :])
                        dots = xp.tile([128, NTMAX], F32, tag="dots")
                        nc.vector.tensor_reduce(
                            dots[:, 0:nt], db[:, 0:nt, :],
                            axis=mybir.AxisListType.X, op=ADD)
                        gc0 = info["tiles"][h2][0][2]
                        nc.vector.tensor_add(dots[:, 0:nt], dots[:, 0:nt],
                                             C["masklog"][:, gc0:gc0 + nt])
                        wh = xp.tile([128, NTMAX], F32, tag="wh")
                        nc.scalar.activation(wh[:, 0:nt], dots[:, 0:nt], Exp,
                                             accum_out=dstage[:, k_sbh:k_sbh + 1])
                        bufs[h2] = rb
                        whs[h2] = wh
                        k_sbh += 1
                    for g in info["groups"]:
                        chain = []
                        for h2 in range(2):
                            for (bt, gg, gc) in info["tiles"][h2]:
                                if gg == g:
                                    chain.append((h2, bt, gc))
                        pg = eps.tile([128, H], F32, tag="pgroup", bufs=2)
                        for i, (h2, bt, gc) in enumerate(chain):
                            oh = ohp.tile([128, 128], F32, tag="oh")
                            nc.vector.tensor_scalar(
                                out=oh[:], in0=C["iota"][:],
                                scalar1=C["collocal"][:, gc:gc + 1],
                                scalar2=whs[h2][:, bt:bt + 1],
                                op0=ISEQ, op1=MUL)
                            nc.tensor.matmul(
                                pg[:], oh[:], bufs[h2][:, bt, H:256],
                                start=(i == 0), stop=(i == len(chain) - 1))
                        nc.vector.tensor_copy(aggr[:, g, :], pg[:])

            NB = 8
            with tc.tile_pool(name=f"node{l}", bufs=2) as npo, \
                 tc.tile_pool(name=f"nps{l}", bufs=1, space="PSUM") as nps:
                # self sims (batched row loads)
                for side in range(2):
                    sl = ag_in if side == 0 else mslice
                    ws = wsu if side == 0 else wsm
                    for b0 in range(0, G, NB):
                        nb = min(NB, G - b0)
                        srows = npo.tile([128, NB, 256], F32, tag="ss_rows")
                        nc.sync.dma_start(
                            srows[:, 0:nb, :],
                            sl[b0 * 128:(b0 + nb) * 128, :].rearrange(
                                "(t p) c -> p t c", p=128))
                        for g in range(b0, b0 + nb):
                            t = g - b0
                            sq = npo.tile([128, H], F32, tag="self_sq")
                            ss = npo.tile([128, 1], F32, tag="self_ss")
                            nc.scalar.activation(sq[:], srows[:, t, 0:H],
                                                 Square, accum_out=ss[:])
                            nc.scalar.activation(ws[:, g:g + 1], ss[:], Exp,
                                                 bias=C["selfmask"][:, g:g + 1])

                # D total + allreduce
                d1 = npo.tile([128, 1], F32, tag="d1")
                nc.vector.tensor_reduce(d1[:], dstage[:],
                                        axis=mybir.AxisListType.X, op=ADD)
                d2 = npo.tile([128, 1], F32, tag="d2")
                nc.vector.tensor_reduce(d2[:], wsu[:],
                                        axis=mybir.AxisListType.X, op=ADD)
                d3 = npo.tile([128, 1], F32, tag="d3")
                nc.vector.tensor_reduce(d3[:], wsm[:],
                                        axis=mybir.AxisListType.X, op=ADD)
                nc.vector.tensor_add(d1[:], d1[:], d2[:])
                nc.vector.tensor_add(d1[:], d1[:], d3[:])
                pd = nps.tile([1, 1], F32, tag="pmisc", bufs=1)
                nc.tensor.matmul(pd[:], d1[:], C["ones_c"][:], start=True,
                                 stop=True)
                dsb = npo.tile([1, 1], F32, tag="dsb")
                nc.vector.tensor_copy(dsb[:], pd[:])
                nc.sync.dma_start(dbounce[0:1, 0:1], dsb[:])
                nc.gpsimd.collective_compute(
                    "AllReduce", ADD, replica_groups=[list(range(NC))],
                    ins=[dbounce[:]], outs=[dout_t[:]])
                dval = npo.tile([1, 1], F32, tag="dval")
                nc.sync.dma_start(dval[:], dout_t[0:1, 0:1])
                pb = nps.tile([128, 1], F32, tag="pmisc", bufs=1)
                nc.tensor.matmul(pb[:], C["ones_r"][:], dval[:], start=True,
                                 stop=True)
                nc.vector.reciprocal(recipD[:], pb[:])

                # ---------------- node update (batched I/O) ----------------
                # v = (lx*ws + aggr)/D + lx  ==  lx*(ws/D + 1) + aggr/D
                wspu = npo.tile([128, G], F32, tag="wspu")
                nc.vector.tensor_scalar(out=wspu[:], in0=wsu[:],
                                        scalar1=recipD[:], scalar2=1.0,
                                        op0=MUL, op1=ADD)
                wspm = npo.tile([128, G], F32, tag="wspm")
                nc.vector.tensor_scalar(out=wspm[:], in0=wsm[:],
                                        scalar1=recipD[:], scalar2=1.0,
                                        op0=MUL, op1=ADD)
                last = (l == L - 1)
                for side in range(2):
                    sl = ag_in if side == 0 else mslice
                    psk = presk_u if side == 0 else presk_m
                    wsp = wspu if side == 0 else wspm
                    for b0 in range(0, G, NB):
                        nb = min(NB, G - b0)
                        rows = npo.tile([128, NB, 256], F32, tag="nd_rows")
                        nc.sync.dma_start(
                            rows[:, 0:nb, :],
                            sl[b0 * 128:(b0 + nb) * 128, :].rearrange(
                                "(t p) c -> p t c", p=128))
                        if l > 0:
                            pkb = npo.tile([128, NB, H], F32, tag="nd_pkb")
                            nc.sync.dma_start(
                                pkb[:, 0:nb, :],
                                psk[b0 * 128:(b0 + nb) * 128, :].rearrange(
                                    "(t p) c -> p t c", p=128))
                        stx = npo.tile([128, NB, 256], F32, tag="nd_stx")
                        stp = npo.tile([128, NB, H], F32, tag="nd_stp")
                        for g in range(b0, b0 + nb):
                            t = g - b0
                            lx = rows[:, t, H:256]
                            v = npo.tile([128, H], F32, tag="nd_v")
                            if side == 0:
                                a2 = npo.tile([128, H], F32, tag="nd_a2")
                                nc.vector.tensor_scalar(
                                    out=a2[:], in0=aggr[:, g, :],
                                    scalar1=recipD[:], scalar2=None, op0=MUL)
                                nc.vector.scalar_tensor_tensor(
                                    out=v[:], in0=lx, scalar=wsp[:, g:g + 1],
                                    in1=a2[:], op0=MUL, op1=ADD)
                            else:
                                nc.vector.tensor_scalar(
                                    out=v[:], in0=lx,
                                    scalar1=wsp[:, g:g + 1],
                                    scalar2=None, op0=MUL)
                            xr = _ln_relu(nc, npo, v, C[f"cg{l}"],
                                          C[f"cb{l}"], do_relu=True)
                            if l > 0:
                                nc.vector.tensor_add(xr[:], xr[:],
                                                     pkb[:, t, :])
                            if not last:
                                finish_node(xr, stx[:, t, 0:H],
                                            C[f"wc{l + 1}T"], C[f"bc{l + 1}"],
                                            stx[:, t, H:256],
                                            [(C[f"ws{l}T"], C[f"bs{l}"],
                                              stp[:, t, :])])
                            else:
                                wext = (C["epuT"] if side == 0 else C["epmT"])
                                bext = (C["epb1"] if side == 0 else None)
                                finish_node(xr, None, None, None, None,
                                            [(wext, bext, stp[:, t, :])])
                        if not last:
                            nc.sync.dma_start(
                                sl[b0 * 128:(b0 + nb) * 128, :].rearrange(
                                    "(t p) c -> p t c", p=128),
                                stx[:, 0:nb, :])
                            nc.sync.dma_start(
                                psk[b0 * 128:(b0 + nb) * 128, :].rearrange(
                                    "(t p) c -> p t c", p=128),
                                stp[:, 0:nb, :])
                        else:
                            c0 = 0 if side == 0 else H
                            nc.sync.dma_start(
                                pp_in[b0 * 128:(b0 + nb) * 128,
                                      c0:c0 + H].rearrange(
                                    "(t p) c -> p t c", p=128),
                                stp[:, 0:nb, :])
            lctx.close()

        # ---------------- final MLP ----------------
        with tc.tile_pool(name="fin", bufs=2) as fp, \
             tc.tile_pool(name="fps", bufs=1, space="PSUM") as fps, \
             tc.tile_pool(name="fout", bufs=2) as fop:
            nc.gpsimd.collective_compute(
                "AllGather", mybir.AluOpType.bypass,
                replica_groups=[list(range(NC))],
                ins=[pp_in[:]], outs=[pptable[:]])
            CHUNK = 4096
            ost = None
            ost_base = 0
            for (s, bt, lo) in sched.fblocks:
                uh = s
                pu = fp.tile([128, 16, H], F32, tag="f_pu")
                nc.gpsimd.dma_gather(
                    out_ap=pu[:, 0:bt, :],
                    in_ap=pptable[uh * HALF:(uh + 1) * HALF, 0:H],
                    idxs_ap=C["fuidx"][:, lo // 16:(lo + bt * 128) // 16],
                    num_idxs=bt * 128, num_idxs_reg=bt * 128,
                    elem_size=H, elem_step=256, single_packet=False)
                pm = fp.tile([128, 16, H], F32, tag="f_pm")
                nc.gpsimd.dma_gather(
                    out_ap=pm[:, 0:bt, :],
                    in_ap=pp_in[:, H:256],
                    idxs_ap=C["fmidx"][:, lo // 16:(lo + bt * 128) // 16],
                    num_idxs=bt * 128, num_idxs_reg=bt * 128,
                    elem_size=H, elem_step=256, single_packet=False)
                h1 = fp.tile([128, 16, H], F32, tag="f_h1")
                nc.vector.tensor_add(h1[:, 0:bt, :], pu[:, 0:bt, :],
                                     pm[:, 0:bt, :])
                h1r = fp.tile([128, 16, H], F32, tag="f_h1r")
                nc.scalar.activation(h1r[:, 0:bt, :], h1[:, 0:bt, :], Relu)
                for t in range(bt):
                    ptr = fps.tile([128, H], F32, tag="f_tr", bufs=2)
                    nc.tensor.transpose(ptr[:], h1r[:, t, :], C["ident"][:])
                    h1T = fp.tile([128, H], F32, tag="f_h1T")
                    nc.scalar.copy(h1T[:], ptr[:])
                    p2 = fps.tile([64, 128], F32, tag="f_p2", bufs=2)
                    nc.tensor.matmul(p2[:], C["ep2T"][:], h1T[:], start=True,
                                     stop=True)
                    h2r = fp.tile([64, 128], F32, tag="f_h2r")
                    nc.scalar.activation(h2r[:], p2[:], Relu,
                                         bias=C["ep2b"][:])
                    p3 = fps.tile([1, 128], F32, tag="f_p3", bufs=1)
                    nc.tensor.matmul(p3[:], C["ep3w"][:], h2r[:], start=True,
                                     stop=True)
                    lane0 = lo + t * 128
                    if ost is None or lane0 - ost_base >= CHUNK:
                        if ost is not None:
                            nc.sync.dma_start(
                                out_dram[0:1, ost_base:ost_base + CHUNK],
                                ost[:])
                        ost = fop.tile([1, CHUNK], F32, tag="f_ost", bufs=1)
                        ost_base = lane0
                        popc = fop.tile([1, CHUNK], F32, tag="f_popc", bufs=1)
                        nc.sync.dma_start(
                            popc[:], inp["fpop"][0:1, ost_base:ost_base + CHUNK])
                        nc.vector.tensor_scalar(
                            out=ost[:], in0=popc[:],
                            scalar1=C["pwpb"][0:1, 0:1],
                            scalar2=C["pwpb"][0:1, 1:2], op0=MUL, op1=ADD)
                    co = lane0 - ost_base
                    nc.vector.tensor_add(ost[0:1, co:co + 128],
                                         ost[0:1, co:co + 128], p3[:])
            if ost is not None:
                nc.sync.dma_start(out_dram[0:1, ost_base:ost_base + CHUNK],
                                  ost[:])

    nc.compile()
    return nc


_CACHE = {}
LAST_RESULT = None


def kernel(x_user, x_movie, edge_index, movie_popularity, params):
    x_user = np.asarray(x_user)
    x_movie = np.asarray(x_movie)
    edge_index = np.asarray(edge_index)
    movie_popularity = np.asarray(movie_popularity)
    sched, in_maps, (E, edge_of_lane) = _prep(
        x_user, x_movie, edge_index, movie_popularity, params)
    nc = _CACHE.get(sched.key)
    if nc is None:
        nc = build(sched)
        _CACHE[sched.key] = nc
    res = run_bass_kernel_spmd(nc, in_maps, core_ids=list(range(NC)))
    global LAST_RESULT
    LAST_RESULT = res
    y = np.zeros((E, 1), np.float32)
    for c in range(NC):
        lanes = edge_of_lane[c]
        m = lanes >= 0
        y[lanes[m], 0] = res.results[c]["out"][0][m]
    return y
